# revision 3
# baseline (speedup 1.0000x reference)
"""Trainium2 Bass kernel: 4x GAT message-passing fusion (gnn_message_passing).

Math reduction used here: the reference GAT has NO nonlinearity between the
edge score e = s_src[src] + s_dst[dst] and the per-dst softmax, so the
s_dst[dst] term (and the segment max m[dst]) cancel inside the softmax:

    alpha_e = exp(s_src[src_e]) / sum_{e'->dst} exp(s_src[src_e'])

Defining per-node g = exp(s_src) and u = g * z, each GAT becomes

    out[dst] = (sum_{e->dst} u[src_e]) / (sum_{e->dst} g[src_e])

i.e. a plain segment-sum of per-node "table" rows [u | g] over incoming
edges, followed by a per-row divide.  No per-edge scalar math at all.

Device plan (8 cores, SPMD):
  Phase A: each core computes its node-shard of the 4 tables
           T = [g*z (d_out) | g | 0-pad]  via bf16 matmuls
           z_aug = h @ [W^T | W^T a_src], g = exp(col d_out).
  AllGather the 4 tables (concatenated per-rank block) -> full table.
  Phase B: edges are sharded by dst; for each 128-dst-node output tile,
           its (padded) incoming edge list is processed in chunks of 128:
           dma_gather 1024 table rows/call, then PE matmul with a host
           precomputed one-hot [128 edge x 128 slot] matrix accumulates
           edge rows into PSUM per dst slot.  Epilogue divides by the
           gathered-g column (guarding isolated nodes) and writes out.
"""

import math
import os
import sys

import numpy as np

for _p in ("/opt/trn_rl_repo", "/opt/trn_rl_repo/concourse"):
    if _p not in sys.path:
        sys.path.insert(0, _p)

import ml_dtypes  # noqa: E402

BF16 = ml_dtypes.bfloat16
P = 128


def _round_up(x, m):
    return (x + m - 1) // m * m


# ----------------------------------------------------------------------------
# Host-side preparation
# ----------------------------------------------------------------------------

def _wrap_idxs(flat, ncalls, idw):
    """[ncalls*idw] -> [P, ncalls*(idw//16)] int16 in dma_gather layout.

    dma_gather reads index i of a call from [partition i%16, col i//16],
    with the 16-partition pattern replicated across the 8 q7 cores.
    """
    idw16 = idw // 16
    a = flat.reshape(ncalls, idw16, 16).transpose(0, 2, 1)  # [ncalls,16,idw16]
    a = np.tile(a, (1, 8, 1))  # [ncalls, 128, idw16]
    return np.ascontiguousarray(a.transpose(1, 0, 2).reshape(P, ncalls * idw16))


def _group_pad(tile_of_edge, srow_of_edge, slot_of_edge, n_tiles, slots_per_tile):
    """Group edges by dst tile, pad each tile to slots_per_tile slots.

    Returns (src_rows [n_tiles, slots_per_tile] int16  (pad -> 0),
             slots    [n_tiles, slots_per_tile] uint8  (pad -> 255))."""
    order = np.argsort(tile_of_edge, kind="stable")
    t_sorted = tile_of_edge[order]
    counts = np.bincount(tile_of_edge, minlength=n_tiles)
    assert counts.max() <= slots_per_tile, (counts.max(), slots_per_tile)
    starts = np.concatenate([[0], np.cumsum(counts)[:-1]])
    pos = np.arange(len(order)) - starts[t_sorted]
    src_pad = np.zeros((n_tiles, slots_per_tile), np.int16)
    slot_pad = np.full((n_tiles, slots_per_tile), 255, np.uint8)
    src_pad[t_sorted, pos] = srow_of_edge[order].astype(np.int16)
    slot_pad[t_sorted, pos] = slot_of_edge[order].astype(np.uint8)
    return src_pad, slot_pad


def _prepare(inputs, R=8, CPC=8):
    """All host-side preprocessing.  Returns (cfg, in_maps, meta)."""
    kn_emb = np.asarray(inputs["kn_emb"], np.float32)
    exer_emb = np.asarray(inputs["exer_emb"], np.float32)
    n_k, d_in = kn_emb.shape
    n_e = exer_emb.shape[0]
    nb = n_e + n_k
    W = {g: np.asarray(inputs[f"W_{g}"], np.float32) for g in ("dir", "und", "kfe", "efk")}
    A = {g: np.asarray(inputs[f"a_{g}"], np.float32) for g in ("dir", "und", "kfe", "efk")}
    d_out = W["dir"].shape[0]

    assert d_in % P == 0 and d_out % 512 == 0
    assert n_k % (R * P) == 0 and n_e % (R * P) == 0 and nb % (R * P) == 0
    KC = d_in // P
    TW = d_out + P                       # [u (d_out) | g | zeros]
    # matmul N slices: 512-wide (one PSUM bank each) + a single column for g
    # (cols d_out+1.. of the table are zero padding; never computed on PE)
    NSL = [(s, s + 512) for s in range(0, d_out, 512)] + [(d_out, d_out + 1)]
    IDW = CPC * P                        # idxs per dma_gather call

    # --- edges: filter + remap --------------------------------------------
    dir_src = np.asarray(inputs["dir_src"], np.int64)
    dir_dst = np.asarray(inputs["dir_dst"], np.int64)
    und_src = np.asarray(inputs["und_src"], np.int64)
    und_dst = np.asarray(inputs["und_dst"], np.int64)
    kfe_src = np.asarray(inputs["kfe_src"], np.int64)
    kfe_dst = np.asarray(inputs["kfe_dst"], np.int64)
    efk_src = np.asarray(inputs["efk_src"], np.int64)
    efk_dst = np.asarray(inputs["efk_dst"], np.int64)

    mk = kfe_dst >= n_e                  # only kn-dst rows of k_from_e are used
    kfe_s, kfe_d = kfe_src[mk], kfe_dst[mk] - n_e
    me = efk_dst < n_e                   # only exer-dst rows of e_from_k are used
    efk_s, efk_d = efk_src[me], efk_dst[me]

    # kfe src compaction (few kn-dst edges -> few distinct src nodes)
    uniq = np.unique(kfe_s) if len(kfe_s) else np.zeros(1, np.int64)
    KCG = _round_up(len(uniq), R * P)
    KCG = min(KCG, _round_up(nb, R * P))
    assert len(uniq) <= KCG
    kfe_c = np.searchsorted(uniq, kfe_s)  # compact src ids

    # --- table layout (per-rank block) ------------------------------------
    SH_KN = n_k // R
    SH_KFE = KCG // R
    SH_EFK = nb // R
    OFF_DIR, OFF_UND = 0, SH_KN
    OFF_KFE = 2 * SH_KN
    OFF_EFK = 2 * SH_KN + SH_KFE
    SH_ROWS = 2 * SH_KN + SH_KFE + SH_EFK
    assert SH_ROWS * R <= 32767, "table rows must fit int16 gather indices"

    def row_dir(n):
        return (n // SH_KN) * SH_ROWS + OFF_DIR + n % SH_KN

    def row_und(n):
        return (n // SH_KN) * SH_ROWS + OFF_UND + n % SH_KN

    def row_kfe(c):
        return (c // SH_KFE) * SH_ROWS + OFF_KFE + c % SH_KFE

    def row_efk(n):
        return (n // SH_EFK) * SH_ROWS + OFF_EFK + n % SH_EFK

    # --- per-graph dst sharding -------------------------------------------
    T_KN = SH_KN // P                    # kn dst tiles per core
    T_EFK = n_e // (R * P)               # exer dst tiles per core

    graphs = []  # (name, src_rows [RT, C*P], slots [RT, C*P], n_tiles/core, C)
    for name, (esrc_row, edst, t_per_core, n_nodes_per_core) in {
        "dir": (row_dir(dir_src), dir_dst, T_KN, SH_KN),
        "und": (row_und(und_src), und_dst, T_KN, SH_KN),
        "kfe": (row_kfe(kfe_c), kfe_d, T_KN, SH_KN),
        "efk": (row_efk(efk_s), efk_d, T_EFK, n_e // R),
    }.items():
        core = edst // n_nodes_per_core
        local = edst % n_nodes_per_core
        gtile = core * t_per_core + local // P
        slot = local % P
        n_tiles_total = R * t_per_core
        cnt = np.bincount(gtile, minlength=n_tiles_total)
        C = max(1, _round_up(int(cnt.max()), P) // P)
        src_pad, slot_pad = _group_pad(gtile, esrc_row, slot, n_tiles_total, C * P)
        graphs.append((name, src_pad, slot_pad, t_per_core, C))

    # --- per-core chunk stream (graph order: dir, und, kfe, efk) ----------
    # chunk_map[i] = (graph_idx, tile_idx_in_core_for_graph, k, C) or None
    chunk_map = []
    for gi, (name, _s, _sl, t_per_core, C) in enumerate(graphs):
        for t in range(t_per_core):
            for k in range(C):
                chunk_map.append((gi, t, k, C))
        while len(chunk_map) % CPC:
            chunk_map.append(None)
    ncalls = len(chunk_map) // CPC
    nchunks = ncalls * CPC

    idx_streams, oh_streams = [], []
    for c in range(R):
        srcs = np.zeros((nchunks, P), np.int16)
        slots = np.full((nchunks, P), 255, np.uint8)
        pos = 0
        for gi, (name, src_pad, slot_pad, t_per_core, C) in enumerate(graphs):
            blk = src_pad[c * t_per_core:(c + 1) * t_per_core].reshape(-1, P)
            srcs[pos:pos + len(blk)] = blk
            blk2 = slot_pad[c * t_per_core:(c + 1) * t_per_core].reshape(-1, P)
            slots[pos:pos + len(blk2)] = blk2
            pos = _round_up(pos + len(blk), CPC)
        idx_streams.append(_wrap_idxs(srcs.reshape(-1), ncalls, IDW))
        oh = (slots[..., None] == np.arange(P, dtype=np.uint8)).astype(BF16)
        oh = oh.reshape(ncalls, CPC, P, P).transpose(0, 2, 1, 3)
        oh_streams.append(np.ascontiguousarray(oh.reshape(ncalls, P, CPC * P)))

    # --- phase A: h blocks + weights --------------------------------------
    kn_bf = kn_emb.astype(BF16)
    ek_bf = np.concatenate([exer_emb.astype(BF16), kn_bf], axis=0)
    KFB = SH_KFE // P
    EFB = SH_EFK // P
    NBLK = T_KN + KFB + EFB

    hT_cores = []
    for c in range(R):
        rows = np.zeros((NBLK * P, d_in), BF16)
        rows[:SH_KN] = kn_bf[c * SH_KN:(c + 1) * SH_KN]
        lo, hi = c * SH_KFE, (c + 1) * SH_KFE
        take = uniq[lo:min(hi, len(uniq))]
        rows[SH_KN:SH_KN + len(take)] = ek_bf[take]
        rows[SH_KN + SH_KFE:] = ek_bf[c * SH_EFK:(c + 1) * SH_EFK]
        hT = rows.reshape(NBLK, P, KC, P).transpose(0, 3, 2, 1)  # [b, p, kc, m]
        hT_cores.append(np.ascontiguousarray(hT.reshape(NBLK, P, KC * P)))

    Wt = np.zeros((4, P, KC * TW), BF16)
    for gi, g in enumerate(("dir", "und", "kfe", "efk")):
        waug = np.zeros((d_in, TW), np.float32)
        waug[:, :d_out] = W[g].T
        waug[:, d_out] = W[g].T @ A[g][:d_out]
        Wt[gi] = waug.astype(BF16).reshape(KC, P, TW).transpose(1, 0, 2).reshape(P, KC * TW)

    # phase A m-tile list: (h_block, graph_id, shard_row0)
    mtiles = []
    for t in range(T_KN):
        mtiles.append((t, 0, OFF_DIR + t * P))
        mtiles.append((t, 1, OFF_UND + t * P))
    for j in range(KFB):
        mtiles.append((T_KN + j, 2, OFF_KFE + j * P))
    for j in range(EFB):
        mtiles.append((T_KN + KFB + j, 3, OFF_EFK + j * P))

    cfg = dict(
        R=R, KC=KC, TW=TW, NSL=NSL, CPC=CPC, IDW=IDW, d_out=d_out,
        SH_ROWS=SH_ROWS, NBLK=NBLK, ncalls=ncalls, nchunks=nchunks,
        mtiles=mtiles, chunk_map=chunk_map, T_KN=T_KN, T_EFK=T_EFK,
        SH_KN=SH_KN, n_kn_graphs=3,
    )
    in_maps = [
        {"hT": hT_cores[c], "Wt": Wt, "idx": idx_streams[c], "oh": oh_streams[c]}
        for c in range(R)
    ]
    meta = dict(n_k=n_k, n_e=n_e, d_out=d_out)
    return cfg, in_maps, meta


# ----------------------------------------------------------------------------
# Device program
# ----------------------------------------------------------------------------

def _build(cfg, debug=False, asserts=False):
    import concourse.bacc as bacc
    import concourse.mybir as mybir
    import concourse.tile as tile
    from concourse.library_config import mlp

    dt = mybir.dt
    AOT = mybir.AluOpType
    R, KC, TW, CPC, IDW = cfg["R"], cfg["KC"], cfg["TW"], cfg["CPC"], cfg["IDW"]
    NSL, d_out = cfg["NSL"], cfg["d_out"]
    SH_ROWS, NBLK, ncalls = cfg["SH_ROWS"], cfg["NBLK"], cfg["ncalls"]
    T_KN, T_EFK, SH_KN = cfg["T_KN"], cfg["T_EFK"], cfg["SH_KN"]
    IDW16 = IDW // 16

    nc = bacc.Bacc("TRN2", target_bir_lowering=False, debug=debug,
                   enable_asserts=asserts, num_devices=R)

    HT = nc.dram_tensor("hT", [NBLK, P, KC * P], dt.bfloat16, kind="ExternalInput")
    WT = nc.dram_tensor("Wt", [4, P, KC * TW], dt.bfloat16, kind="ExternalInput")
    IDX = nc.dram_tensor("idx", [P, ncalls * IDW16], dt.int16, kind="ExternalInput")
    OH = nc.dram_tensor("oh", [ncalls, P, CPC * P], dt.bfloat16, kind="ExternalInput")
    KN_OUT = nc.dram_tensor("kn_out", [SH_KN, d_out], dt.float32, kind="ExternalOutput")
    EX_OUT = nc.dram_tensor("ex_out", [T_EFK * P, d_out], dt.float32, kind="ExternalOutput")

    with tile.TileContext(nc) as tc:
        nc.gpsimd.load_library(mlp)
        with tc.tile_pool(name="dram", bufs=1, space="DRAM") as dramp:
            shard = dramp.tile([SH_ROWS, TW], dt.bfloat16, name="shard_tab")
            table = dramp.tile([SH_ROWS * R, TW], dt.bfloat16,
                               addr_space="Shared", name="full_tab")

            # ---------------- Phase A: build table shard ----------------
            with (
                tc.tile_pool(name="wp", bufs=2) as wp,
                tc.tile_pool(name="hp", bufs=3) as hp,
                tc.tile_pool(name="psA", bufs=2, space="PSUM") as psA,
                tc.tile_pool(name="tabp", bufs=3) as tabp,
                tc.tile_pool(name="gsp", bufs=4) as gsp,
            ):
                w_sb = {}
                h_sb, cur_blk = None, None
                for (b, g, r0) in cfg["mtiles"]:
                    if g not in w_sb:
                        # pool bufs=2 keeps the two live graphs' W resident;
                        # older allocations' slots are recycled by the pool
                        w = wp.tile([P, KC * TW], dt.bfloat16, tag="w", name=f"w{g}")
                        nc.sync.dma_start(out=w[:], in_=WT.ap()[g])
                        w_sb[g] = w
                    if b != cur_blk:
                        h_sb = hp.tile([P, KC * P], dt.bfloat16, tag="h", name=f"h{b}")
                        nc.sync.dma_start(out=h_sb[:], in_=HT.ap()[b])
                        cur_blk = b
                    w = w_sb[g]
                    ps = psA.tile([P, TW], dt.float32, tag="psA", name=f"psA{b}_{g}")
                    for (s0, s1) in NSL:
                        for kc in range(KC):
                            nc.tensor.matmul(
                                out=ps[:, s0:s1],
                                lhsT=h_sb[:, kc * P:(kc + 1) * P],
                                rhs=w[:, kc * TW + s0:kc * TW + s1],
                                start=(kc == 0), stop=(kc == KC - 1),
                            )
                    gv = gsp.tile([P, 1], dt.float32, tag="gv", name=f"gv{b}_{g}")
                    nc.scalar.activation(gv[:, :1], ps[:, d_out:d_out + 1],
                                         mybir.ActivationFunctionType.Exp)
                    tab = tabp.tile([P, TW], dt.bfloat16, tag="tab", name=f"tab{b}_{g}")
                    nc.vector.tensor_scalar(tab[:, 0:d_out], ps[:, 0:d_out],
                                            gv[:, :1], None, AOT.mult)
                    nc.vector.tensor_copy(out=tab[:, d_out:d_out + 1], in_=gv[:, :1])
                    nc.vector.memset(tab[:, d_out + 1:TW], 0.0)
                    nc.sync.dma_start(out=shard[r0:r0 + P, :], in_=tab[:])

            # ---------------- AllGather ----------------
            nc.gpsimd.collective_compute(
                "AllGather", AOT.bypass,
                replica_groups=[list(range(R))],
                ins=[shard.opt()], outs=[table.opt()],
            )

            # ---------------- Phase B: gather + aggregate ----------------
            with (
                tc.tile_pool(name="idxp", bufs=1) as idxp,
                tc.tile_pool(name="gp", bufs=3) as gp,
                tc.tile_pool(name="ohp", bufs=3) as ohp,
                tc.tile_pool(name="psB", bufs=2, space="PSUM") as psB,
                tc.tile_pool(name="accp", bufs=max(1, T_KN)) as accp,
                tc.tile_pool(name="outp", bufs=3) as outp,
                tc.tile_pool(name="eps", bufs=8) as eps,
            ):
                idx_sb = idxp.tile([P, ncalls * IDW16], dt.int16, name="idx_sb")
                nc.sync.dma_start(out=idx_sb[:], in_=IDX.ap()[:, :])

                acc = {}     # kn tile -> acc sbuf tile
                cur_ps = {}  # (graph,tile) currently accumulating
                n_kn_graphs = cfg["n_kn_graphs"]

                def epilogue(gi, t, ps):
                    den = eps.tile([P, 1], dt.float32, tag="den", name=f"den{gi}_{t}")
                    z0 = eps.tile([P, 1], dt.float32, tag="z0", name=f"z0{gi}_{t}")
                    nc.vector.tensor_scalar(z0[:, :1], ps[:, d_out:d_out + 1],
                                            0.0, None, AOT.is_equal)
                    nc.vector.tensor_tensor(out=den[:, :1], in0=ps[:, d_out:d_out + 1],
                                            in1=z0[:, :1], op=AOT.add)
                    rec = eps.tile([P, 1], dt.float32, tag="rec", name=f"rec{gi}_{t}")
                    nc.vector.reciprocal(rec[:, :1], den[:, :1])
                    if gi == 0:
                        a = accp.tile([P, d_out], dt.float32, tag="acc", name=f"acc{t}")
                        acc[t] = a
                        nc.vector.tensor_scalar(a[:], ps[:, 0:d_out], rec[:, :1],
                                                None, AOT.mult)
                    elif gi < n_kn_graphs:
                        tmp = outp.tile([P, d_out], dt.float32, tag="o", name=f"tmp{gi}_{t}")
                        nc.vector.tensor_scalar(tmp[:], ps[:, 0:d_out], rec[:, :1],
                                                None, AOT.mult)
                        nc.vector.tensor_tensor(out=acc[t][:], in0=acc[t][:],
                                                in1=tmp[:], op=AOT.add)
                        if gi == n_kn_graphs - 1:
                            nc.sync.dma_start(out=KN_OUT.ap()[t * P:(t + 1) * P, :],
                                              in_=acc[t][:])
                    else:
                        o = outp.tile([P, d_out], dt.float32, tag="o", name=f"o{t}")
                        nc.vector.tensor_scalar(o[:], ps[:, 0:d_out], rec[:, :1],
                                                None, AOT.mult)
                        nc.sync.dma_start(out=EX_OUT.ap()[t * P:(t + 1) * P, :], in_=o[:])

                for call in range(ncalls):
                    gt = gp.tile([P, CPC, TW], dt.bfloat16, tag="gt", name=f"gt{call}")
                    nc.gpsimd.dma_gather(
                        gt[:], table[:, :],
                        idx_sb[:, call * IDW16:(call + 1) * IDW16],
                        IDW, IDW, TW,
                    )
                    oh_sb = ohp.tile([P, CPC * P], dt.bfloat16, tag="oh", name=f"oh{call}")
                    nc.sync.dma_start(out=oh_sb[:], in_=OH.ap()[call])
                    for c in range(CPC):
                        cm = cfg["chunk_map"][call * CPC + c]
                        if cm is None:
                            continue
                        gi, t, k, C = cm
                        if k == 0:
                            ps = psB.tile([P, TW], dt.float32, tag="psB",
                                          name=f"psB{gi}_{t}")
                            cur_ps[(gi, t)] = ps
                        ps = cur_ps[(gi, t)]
                        for (s0, s1) in NSL:
                            nc.tensor.matmul(
                                out=ps[:, s0:s1],
                                lhsT=oh_sb[:, c * P:(c + 1) * P],
                                rhs=gt[:, c, s0:s1],
                                start=(k == 0), stop=(k == C - 1),
                            )
                        if k == C - 1:
                            epilogue(gi, t, ps)

    nc.compile()
    return nc


# ----------------------------------------------------------------------------
# Entry point
# ----------------------------------------------------------------------------

_CACHE = {}


def _run(inputs, R=8, sim=False):
    cfg, in_maps, meta = _prepare(inputs, R=R)

    key = (R, cfg["ncalls"], cfg["NBLK"], cfg["SH_ROWS"], cfg["TW"], cfg["KC"],
           tuple(x if x is None else x[:3] for x in cfg["chunk_map"]), sim)
    if key in _CACHE:
        nc = _CACHE[key]
    else:
        nc = _build(cfg, debug=sim, asserts=sim)
        _CACHE[key] = nc

    if sim:
        from concourse.bass_interp import MultiCoreSim
        msim = MultiCoreSim(nc, num_cores=R)
        for c in range(R):
            for k, v in in_maps[c].items():
                msim.cores[c].tensor(k)[:] = v
        msim.simulate(check_with_hw=False)
        results = [
            {"kn_out": np.array(msim.cores[c].tensor("kn_out")),
             "ex_out": np.array(msim.cores[c].tensor("ex_out"))}
            for c in range(R)
        ]
        exec_ns = None
    else:
        from concourse.bass_utils import run_bass_kernel_spmd
        trace = bool(int(os.environ.get("KERNEL_TRACE", "0")))
        br = run_bass_kernel_spmd(nc, in_maps, list(range(R)), trace=trace)
        results = br.results
        exec_ns = br.exec_time_ns

    n_k, n_e, d_out = meta["n_k"], meta["n_e"], meta["d_out"]
    kn_out = np.concatenate([results[c]["kn_out"] for c in range(R)], axis=0)
    ex_out = np.concatenate([results[c]["ex_out"] for c in range(R)], axis=0)
    assert kn_out.shape == (n_k, d_out) and ex_out.shape == (n_e, d_out)
    return (np.asarray(kn_out, np.float32), np.asarray(ex_out, np.float32)), exec_ns


def kernel(**inputs):
    out, _ = _run(inputs, R=8, sim=False)
    return out


def kernel_timed(**inputs):
    return _run(inputs, R=8, sim=False)


def kernel_sim(R=2, **inputs):
    out, _ = _run(inputs, R=R, sim=True)
    return out


# revision 13
# speedup vs baseline: 1.0979x; 1.0979x over previous
"""Trainium2 Bass kernel: 4x GAT message-passing fusion (gnn_message_passing).

Math reduction used here: the reference GAT has NO nonlinearity between the
edge score e = s_src[src] + s_dst[dst] and the per-dst softmax, so the
s_dst[dst] term (and the segment max m[dst]) cancel inside the softmax:

    alpha_e = exp(s_src[src_e]) / sum_{e'->dst} exp(s_src[src_e'])

Defining per-node g = exp(s_src) and u = g * z, each GAT becomes

    out[dst] = (sum_{e->dst} u[src_e]) / (sum_{e->dst} g[src_e])

i.e. a plain segment-sum of per-node "table" rows [u | g] over incoming
edges, followed by a per-row divide.  No per-edge scalar math at all.

Device plan (8 cores, SPMD):
  Phase A: each core computes its node-shard of the 4 tables
           T = [g*z (d_out) | g | 0-pad]  via bf16 matmuls
           z_aug = h @ [W^T | W^T a_src], g = exp(col d_out).
  AllGather the 4 tables (concatenated per-rank block) -> full table.
  Phase B: edges are sharded by dst; for each 128-dst-node output tile,
           its (padded) incoming edge list is processed in chunks of 128:
           dma_gather 1024 table rows/call, then PE matmul with a host
           precomputed one-hot [128 edge x 128 slot] matrix accumulates
           edge rows into PSUM per dst slot.  Epilogue divides by the
           gathered-g column (guarding isolated nodes) and writes out.
"""

import math
import os
import sys

import numpy as np

for _p in ("/opt/trn_rl_repo", "/opt/trn_rl_repo/concourse"):
    if _p not in sys.path:
        sys.path.insert(0, _p)

import ml_dtypes  # noqa: E402

BF16 = ml_dtypes.bfloat16
P = 128


def _round_up(x, m):
    return (x + m - 1) // m * m


# ----------------------------------------------------------------------------
# Host-side preparation
# ----------------------------------------------------------------------------

def _wrap_idxs(flat, ncalls, idw):
    """[ncalls*idw] -> [P, ncalls*(idw//16)] int16 in dma_gather layout.

    dma_gather reads index i of a call from [partition i%16, col i//16],
    with the 16-partition pattern replicated across the 8 q7 cores.
    """
    idw16 = idw // 16
    a = flat.reshape(ncalls, idw16, 16).transpose(0, 2, 1)  # [ncalls,16,idw16]
    a = np.tile(a, (1, 8, 1))  # [ncalls, 128, idw16]
    return np.ascontiguousarray(a.transpose(1, 0, 2).reshape(P, ncalls * idw16))


def _group_pad(tile_of_edge, srow_of_edge, slot_of_edge, n_tiles, slots_per_tile):
    """Group edges by dst tile, pad each tile to slots_per_tile slots.

    Returns (src_rows [n_tiles, slots_per_tile] int16  (pad -> 0),
             slots    [n_tiles, slots_per_tile] uint8  (pad -> 255))."""
    order = np.argsort(tile_of_edge, kind="stable")
    t_sorted = tile_of_edge[order]
    counts = np.bincount(tile_of_edge, minlength=n_tiles)
    assert counts.max() <= slots_per_tile, (counts.max(), slots_per_tile)
    starts = np.concatenate([[0], np.cumsum(counts)[:-1]])
    pos = np.arange(len(order)) - starts[t_sorted]
    src_pad = np.zeros((n_tiles, slots_per_tile), np.int16)
    slot_pad = np.full((n_tiles, slots_per_tile), 255, np.uint8)
    src_pad[t_sorted, pos] = srow_of_edge[order].astype(np.int16)
    slot_pad[t_sorted, pos] = slot_of_edge[order].astype(np.uint8)
    return src_pad, slot_pad


def _prepare(inputs, R=8, CPC=8):
    """All host-side preprocessing.  Returns (cfg, in_maps, meta)."""
    kn_emb = np.asarray(inputs["kn_emb"], np.float32)
    exer_emb = np.asarray(inputs["exer_emb"], np.float32)
    n_k, d_in = kn_emb.shape
    n_e = exer_emb.shape[0]
    nb = n_e + n_k
    W = {g: np.asarray(inputs[f"W_{g}"], np.float32) for g in ("dir", "und", "kfe", "efk")}
    A = {g: np.asarray(inputs[f"a_{g}"], np.float32) for g in ("dir", "und", "kfe", "efk")}
    d_out = W["dir"].shape[0]

    assert d_in % P == 0 and d_out % 512 == 0
    assert n_k % (R * P) == 0 and n_e % (R * P) == 0 and nb % (R * P) == 0
    KC = d_in // P
    TW = d_out + P                       # [u (d_out) | g | zeros]
    # matmul N slices: 512-wide (one PSUM bank each) + a single column for g
    # (cols d_out+1.. of the table are zero padding; never computed on PE)
    NSL = [(s, s + 512) for s in range(0, d_out, 512)] + [(d_out, d_out + 1)]
    IDW = CPC * P                        # idxs per dma_gather call

    # --- edges: filter + remap --------------------------------------------
    dir_src = np.asarray(inputs["dir_src"], np.int64)
    dir_dst = np.asarray(inputs["dir_dst"], np.int64)
    und_src = np.asarray(inputs["und_src"], np.int64)
    und_dst = np.asarray(inputs["und_dst"], np.int64)
    kfe_src = np.asarray(inputs["kfe_src"], np.int64)
    kfe_dst = np.asarray(inputs["kfe_dst"], np.int64)
    efk_src = np.asarray(inputs["efk_src"], np.int64)
    efk_dst = np.asarray(inputs["efk_dst"], np.int64)

    mk = kfe_dst >= n_e                  # only kn-dst rows of k_from_e are used
    kfe_s, kfe_d = kfe_src[mk], kfe_dst[mk] - n_e
    me = efk_dst < n_e                   # only exer-dst rows of e_from_k are used
    efk_s, efk_d = efk_src[me], efk_dst[me]

    # kfe src compaction (few kn-dst edges -> few distinct src nodes)
    uniq = np.unique(kfe_s) if len(kfe_s) else np.zeros(1, np.int64)
    KCG = _round_up(len(uniq), R * P)
    KCG = min(KCG, _round_up(nb, R * P))
    assert len(uniq) <= KCG
    kfe_c = np.searchsorted(uniq, kfe_s)  # compact src ids

    # --- table layout (per-rank block) ------------------------------------
    # Two gathered tables:
    #   tab_kn  (one AllGather): per-rank block [dir SH_KN | und SH_KN | kfe SH_KFE]
    #   tab_efk (banded AllGathers): efk rows in NBAND row-bands so the
    #   all-gather of band b can fire as soon as every rank finished band b.
    SH_KN = n_k // R
    SH_KFE = KCG // R
    SH_EFK = nb // R
    KN_BLOCK = 2 * SH_KN + SH_KFE        # per-rank rows in tab_kn
    SH_ROWS = KN_BLOCK + SH_EFK          # per-rank shard rows (bounce layout)
    assert KN_BLOCK * R <= 32767 and SH_EFK * R <= 32767

    EFB = SH_EFK // P                    # efk h-blocks per core
    # NRT shared-output tensors allow a single collective writer, so the efk
    # table is one AllGather (NBAND=1); the kn table's separate early AG still
    # lets phase-B kn work overlap the efk matmuls and the big efk AG.
    NBAND = 1
    BAND_BLOCKS = EFB // NBAND           # h-blocks per band
    BAND_ROWS = BAND_BLOCKS * P          # rows per rank per band

    def row_dir(n):
        return (n // SH_KN) * KN_BLOCK + n % SH_KN

    def row_und(n):
        return (n // SH_KN) * KN_BLOCK + SH_KN + n % SH_KN

    def row_kfe(c):
        return (c // SH_KFE) * KN_BLOCK + 2 * SH_KN + c % SH_KFE

    def row_efk(n):
        rank, local = n // SH_EFK, n % SH_EFK
        band, bl = local // BAND_ROWS, local % BAND_ROWS
        return band * (R * BAND_ROWS) + rank * BAND_ROWS + bl

    # --- per-graph dst sharding -------------------------------------------
    T_KN = SH_KN // P                    # kn dst tiles per core
    T_EFK = n_e // (R * P)               # exer dst tiles per core

    graphs = []  # (name, src_rows [RT, C*P], slots [RT, C*P], n_tiles/core, C)
    for name, (esrc_row, edst, t_per_core, n_nodes_per_core) in {
        "dir": (row_dir(dir_src), dir_dst, T_KN, SH_KN),
        "und": (row_und(und_src), und_dst, T_KN, SH_KN),
        "kfe": (row_kfe(kfe_c), kfe_d, T_KN, SH_KN),
        "efk": (row_efk(efk_s), efk_d, T_EFK, n_e // R),
    }.items():
        core = edst // n_nodes_per_core
        local = edst % n_nodes_per_core
        gtile = core * t_per_core + local // P
        slot = local % P
        n_tiles_total = R * t_per_core
        cnt = np.bincount(gtile, minlength=n_tiles_total)
        C = max(1, _round_up(int(cnt.max()), P) // P)
        src_pad, slot_pad = _group_pad(gtile, esrc_row, slot, n_tiles_total, C * P)
        graphs.append((name, src_pad, slot_pad, t_per_core, C))

    # --- per-core chunk stream (graph order: dir, und, kfe, efk) ----------
    # chunk_map[i] = (graph_idx, tile_idx_in_core_for_graph, k, C) or None
    # kn-group calls (dir/und/kfe -> gather from tab_kn) come first, then the
    # efk calls (gather from tab_efk); each group starts on a call boundary.
    chunk_map = []
    for gi, (name, _s, _sl, t_per_core, C) in enumerate(graphs):
        for t in range(t_per_core):
            for k in range(C):
                chunk_map.append((gi, t, k, C))
        while len(chunk_map) % CPC:
            chunk_map.append(None)
        if name == "kfe":
            ncalls_kn = len(chunk_map) // CPC
    ncalls = len(chunk_map) // CPC
    nchunks = ncalls * CPC

    idx_streams, oh_streams = [], []
    for c in range(R):
        srcs = np.zeros((nchunks, P), np.int16)
        slots = np.full((nchunks, P), 255, np.uint8)
        pos = 0
        for gi, (name, src_pad, slot_pad, t_per_core, C) in enumerate(graphs):
            blk = src_pad[c * t_per_core:(c + 1) * t_per_core].reshape(-1, P)
            srcs[pos:pos + len(blk)] = blk
            blk2 = slot_pad[c * t_per_core:(c + 1) * t_per_core].reshape(-1, P)
            slots[pos:pos + len(blk2)] = blk2
            pos = _round_up(pos + len(blk), CPC)
        idx_streams.append(_wrap_idxs(srcs.reshape(-1), ncalls, IDW))
        oh = (slots[..., None] == np.arange(P, dtype=np.uint8)).astype(BF16)
        oh = oh.reshape(ncalls, CPC, P, P).transpose(0, 2, 1, 3)
        oh_streams.append(np.ascontiguousarray(oh.reshape(ncalls, P, CPC * P)))

    # --- phase A: h blocks + weights --------------------------------------
    kn_bf = kn_emb.astype(BF16)
    ek_bf = np.concatenate([exer_emb.astype(BF16), kn_bf], axis=0)
    KFB = SH_KFE // P
    EFB = SH_EFK // P
    NBLK = T_KN + KFB + EFB

    hT_cores = []
    for c in range(R):
        rows = np.zeros((NBLK * P, d_in), BF16)
        rows[:SH_KN] = kn_bf[c * SH_KN:(c + 1) * SH_KN]
        lo, hi = c * SH_KFE, (c + 1) * SH_KFE
        take = uniq[lo:min(hi, len(uniq))]
        rows[SH_KN:SH_KN + len(take)] = ek_bf[take]
        rows[SH_KN + SH_KFE:] = ek_bf[c * SH_EFK:(c + 1) * SH_EFK]
        hT = rows.reshape(NBLK, P, KC, P).transpose(0, 3, 2, 1)  # [b, p, kc, m]
        hT_cores.append(np.ascontiguousarray(hT.reshape(NBLK, P, KC * P)))

    Wt = np.zeros((4, P, KC * TW), BF16)
    for gi, g in enumerate(("dir", "und", "kfe", "efk")):
        waug = np.zeros((d_in, TW), np.float32)
        waug[:, :d_out] = W[g].T
        waug[:, d_out] = W[g].T @ A[g][:d_out]
        Wt[gi] = waug.astype(BF16).reshape(KC, P, TW).transpose(1, 0, 2).reshape(P, KC * TW)

    # phase A m-tile list: (h_block, graph_id, shard_row0)
    mtiles = []
    for t in range(T_KN):
        mtiles.append((t, 0, t * P))
        mtiles.append((t, 1, SH_KN + t * P))
    for j in range(KFB):
        mtiles.append((T_KN + j, 2, 2 * SH_KN + j * P))
    for j in range(EFB):
        mtiles.append((T_KN + KFB + j, 3, KN_BLOCK + j * P))

    cfg = dict(
        R=R, KC=KC, TW=TW, NSL=NSL, CPC=CPC, IDW=IDW, d_out=d_out,
        SH_ROWS=SH_ROWS, NBLK=NBLK, ncalls=ncalls, nchunks=nchunks,
        ncalls_kn=ncalls_kn, KN_BLOCK=KN_BLOCK, NBAND=NBAND,
        BAND_ROWS=BAND_ROWS, SH_EFK=SH_EFK,
        mtiles=mtiles, chunk_map=chunk_map, T_KN=T_KN, T_EFK=T_EFK,
        SH_KN=SH_KN, n_kn_graphs=3,
    )
    in_maps = [
        {"hT": hT_cores[c], "Wt": Wt, "idx": idx_streams[c], "oh": oh_streams[c]}
        for c in range(R)
    ]
    meta = dict(n_k=n_k, n_e=n_e, d_out=d_out)
    return cfg, in_maps, meta


# ----------------------------------------------------------------------------
# Device program
# ----------------------------------------------------------------------------

def _build(cfg, debug=False, asserts=False):
    import concourse.bacc as bacc
    import concourse.mybir as mybir
    import concourse.tile as tile
    from concourse.library_config import mlp

    dt = mybir.dt
    AOT = mybir.AluOpType
    R, KC, TW, CPC, IDW = cfg["R"], cfg["KC"], cfg["TW"], cfg["CPC"], cfg["IDW"]
    NSL, d_out = cfg["NSL"], cfg["d_out"]
    SH_ROWS, NBLK, ncalls = cfg["SH_ROWS"], cfg["NBLK"], cfg["ncalls"]
    T_KN, T_EFK, SH_KN = cfg["T_KN"], cfg["T_EFK"], cfg["SH_KN"]
    IDW16 = IDW // 16

    ncalls_kn = cfg["ncalls_kn"]
    KN_BLOCK, NBAND = cfg["KN_BLOCK"], cfg["NBAND"]
    BAND_ROWS, SH_EFK = cfg["BAND_ROWS"], cfg["SH_EFK"]

    nc = bacc.Bacc("TRN2", target_bir_lowering=False, debug=debug,
                   enable_asserts=asserts, num_devices=R)

    HT = nc.dram_tensor("hT", [NBLK, P, KC * P], dt.bfloat16, kind="ExternalInput")
    WT = nc.dram_tensor("Wt", [4, P, KC * TW], dt.bfloat16, kind="ExternalInput")
    IDX = nc.dram_tensor("idx", [P, ncalls * IDW16], dt.int16, kind="ExternalInput")
    OH = nc.dram_tensor("oh", [ncalls, P, CPC * P], dt.bfloat16, kind="ExternalInput")
    KN_OUT = nc.dram_tensor("kn_out", [SH_KN, d_out], dt.float32, kind="ExternalOutput")
    EX_OUT = nc.dram_tensor("ex_out", [T_EFK * P, d_out], dt.float32, kind="ExternalOutput")

    with tile.TileContext(nc) as tc:
        nc.gpsimd.load_library(mlp)
        with tc.tile_pool(name="dram", bufs=1, space="DRAM") as dramp:
            shard = dramp.tile([SH_ROWS, TW], dt.bfloat16, name="shard_tab")
            tab_kn = dramp.tile([KN_BLOCK * R, TW], dt.bfloat16,
                                addr_space="Shared", name="tab_kn")
            tab_efk = dramp.tile([SH_EFK * R, TW], dt.bfloat16,
                                 addr_space="Shared", name="tab_efk")
            rg = [list(range(R))]

            def ag(shard_r0, nrows, out_tab, out_r0):
                nc.gpsimd.collective_compute(
                    "AllGather", AOT.bypass, replica_groups=rg,
                    ins=[shard[shard_r0:shard_r0 + nrows, :]],
                    outs=[out_tab[out_r0:out_r0 + nrows * R, :]],
                )

            # ---------------- Phase A: build table shard ----------------
            with (
                tc.tile_pool(name="wp", bufs=2) as wp,
                tc.tile_pool(name="hp", bufs=3) as hp,
                tc.tile_pool(name="psA", bufs=2, space="PSUM") as psA,
                tc.tile_pool(name="tabp", bufs=3) as tabp,
                tc.tile_pool(name="gsp", bufs=4) as gsp,
            ):
                n_kn_mtiles = sum(1 for (_b, g, _r) in cfg["mtiles"] if g < 3)
                efk_done = 0
                w_sb = {}
                h_sb, cur_blk = None, None
                for mi, (b, g, r0) in enumerate(cfg["mtiles"]):
                    if g not in w_sb:
                        # pool bufs=2 keeps the two live graphs' W resident;
                        # older allocations' slots are recycled by the pool
                        w = wp.tile([P, KC * TW], dt.bfloat16, tag="w", name=f"w{g}")
                        nc.sync.dma_start(out=w[:], in_=WT.ap()[g])
                        w_sb[g] = w
                    if b != cur_blk:
                        h_sb = hp.tile([P, KC * P], dt.bfloat16, tag="h", name=f"h{b}")
                        nc.sync.dma_start(out=h_sb[:], in_=HT.ap()[b])
                        cur_blk = b
                    w = w_sb[g]
                    ps = psA.tile([P, TW], dt.float32, tag="psA", name=f"psA{b}_{g}")
                    for (s0, s1) in NSL:
                        for kc in range(KC):
                            nc.tensor.matmul(
                                out=ps[:, s0:s1],
                                lhsT=h_sb[:, kc * P:(kc + 1) * P],
                                rhs=w[:, kc * TW + s0:kc * TW + s1],
                                start=(kc == 0), stop=(kc == KC - 1),
                            )
                    gv = gsp.tile([P, 1], dt.float32, tag="gv", name=f"gv{b}_{g}")
                    nc.scalar.activation(gv[:, :1], ps[:, d_out:d_out + 1],
                                         mybir.ActivationFunctionType.Exp)
                    tab = tabp.tile([P, TW], dt.bfloat16, tag="tab", name=f"tab{b}_{g}")
                    nc.vector.tensor_scalar(tab[:, 0:d_out], ps[:, 0:d_out],
                                            gv[:, :1], None, AOT.mult)
                    nc.vector.tensor_copy(out=tab[:, d_out:d_out + 1], in_=gv[:, :1])
                    nc.vector.memset(tab[:, d_out + 1:TW], 0.0)
                    nc.sync.dma_start(out=shard[r0:r0 + P, :], in_=tab[:])
                    # fire the all-gathers as soon as their band is complete
                    if mi == n_kn_mtiles - 1:
                        ag(0, KN_BLOCK, tab_kn, 0)
                    if g == 3:
                        efk_done += 1
                        if efk_done % (BAND_ROWS // P) == 0:
                            band = efk_done // (BAND_ROWS // P) - 1
                            ag(KN_BLOCK + band * BAND_ROWS, BAND_ROWS,
                               tab_efk, band * BAND_ROWS * R)

            # ---------------- Phase B: gather + aggregate ----------------
            with (
                tc.tile_pool(name="idxp", bufs=1) as idxp,
                tc.tile_pool(name="gp", bufs=3) as gp,
                tc.tile_pool(name="ohp", bufs=3) as ohp,
                tc.tile_pool(name="psB", bufs=2, space="PSUM") as psB,
                tc.tile_pool(name="accp", bufs=max(1, T_KN)) as accp,
                tc.tile_pool(name="outp", bufs=3) as outp,
                tc.tile_pool(name="eps", bufs=8) as eps,
            ):
                idx_sb = idxp.tile([P, ncalls * IDW16], dt.int16, name="idx_sb")
                nc.sync.dma_start(out=idx_sb[:], in_=IDX.ap()[:, :])

                acc = {}     # kn tile -> acc sbuf tile
                cur_ps = {}  # (graph,tile) currently accumulating
                n_kn_graphs = cfg["n_kn_graphs"]

                def epilogue(gi, t, ps):
                    den = eps.tile([P, 1], dt.float32, tag="den", name=f"den{gi}_{t}")
                    z0 = eps.tile([P, 1], dt.float32, tag="z0", name=f"z0{gi}_{t}")
                    nc.vector.tensor_scalar(z0[:, :1], ps[:, d_out:d_out + 1],
                                            0.0, None, AOT.is_equal)
                    nc.vector.tensor_tensor(out=den[:, :1], in0=ps[:, d_out:d_out + 1],
                                            in1=z0[:, :1], op=AOT.add)
                    rec = eps.tile([P, 1], dt.float32, tag="rec", name=f"rec{gi}_{t}")
                    nc.vector.reciprocal(rec[:, :1], den[:, :1])
                    if gi == 0:
                        a = accp.tile([P, d_out], dt.float32, tag="acc", name=f"acc{t}")
                        acc[t] = a
                        nc.vector.tensor_scalar(a[:], ps[:, 0:d_out], rec[:, :1],
                                                None, AOT.mult)
                    elif gi < n_kn_graphs:
                        tmp = outp.tile([P, d_out], dt.float32, tag="o", name=f"tmp{gi}_{t}")
                        nc.vector.tensor_scalar(tmp[:], ps[:, 0:d_out], rec[:, :1],
                                                None, AOT.mult)
                        nc.vector.tensor_tensor(out=acc[t][:], in0=acc[t][:],
                                                in1=tmp[:], op=AOT.add)
                        if gi == n_kn_graphs - 1:
                            nc.sync.dma_start(out=KN_OUT.ap()[t * P:(t + 1) * P, :],
                                              in_=acc[t][:])
                    else:
                        o = outp.tile([P, d_out], dt.float32, tag="o", name=f"o{t}")
                        nc.vector.tensor_scalar(o[:], ps[:, 0:d_out], rec[:, :1],
                                                None, AOT.mult)
                        nc.sync.dma_start(out=EX_OUT.ap()[t * P:(t + 1) * P, :], in_=o[:])

                for call in range(ncalls):
                    src_tab = tab_kn if call < ncalls_kn else tab_efk
                    gt = gp.tile([P, CPC, TW], dt.bfloat16, tag="gt", name=f"gt{call}")
                    nc.gpsimd.dma_gather(
                        gt[:], src_tab[:, :],
                        idx_sb[:, call * IDW16:(call + 1) * IDW16],
                        IDW, IDW, TW,
                    )
                    oh_sb = ohp.tile([P, CPC * P], dt.bfloat16, tag="oh", name=f"oh{call}")
                    nc.sync.dma_start(out=oh_sb[:], in_=OH.ap()[call])
                    for c in range(CPC):
                        cm = cfg["chunk_map"][call * CPC + c]
                        if cm is None:
                            continue
                        gi, t, k, C = cm
                        if k == 0:
                            ps = psB.tile([P, TW], dt.float32, tag="psB",
                                          name=f"psB{gi}_{t}")
                            cur_ps[(gi, t)] = ps
                        ps = cur_ps[(gi, t)]
                        for (s0, s1) in NSL:
                            nc.tensor.matmul(
                                out=ps[:, s0:s1],
                                lhsT=oh_sb[:, c * P:(c + 1) * P],
                                rhs=gt[:, c, s0:s1],
                                start=(k == 0), stop=(k == C - 1),
                            )
                        if k == C - 1:
                            epilogue(gi, t, ps)

    nc.compile()
    return nc


# ----------------------------------------------------------------------------
# Entry point
# ----------------------------------------------------------------------------

_CACHE = {}


def _run(inputs, R=8, sim=False):
    cfg, in_maps, meta = _prepare(inputs, R=R)

    key = (R, cfg["ncalls"], cfg["ncalls_kn"], cfg["NBAND"], cfg["NBLK"],
           cfg["SH_ROWS"], cfg["TW"], cfg["KC"],
           tuple(x if x is None else x[:3] for x in cfg["chunk_map"]), sim)
    if key in _CACHE:
        nc = _CACHE[key]
    else:
        nc = _build(cfg, debug=sim, asserts=sim)
        _CACHE[key] = nc

    if sim:
        from concourse.bass_interp import MultiCoreSim
        msim = MultiCoreSim(nc, num_cores=R)
        for c in range(R):
            for k, v in in_maps[c].items():
                msim.cores[c].tensor(k)[:] = v
        msim.simulate(check_with_hw=False)
        results = [
            {"kn_out": np.array(msim.cores[c].tensor("kn_out")),
             "ex_out": np.array(msim.cores[c].tensor("ex_out"))}
            for c in range(R)
        ]
        exec_ns = None
    else:
        from concourse.bass_utils import run_bass_kernel_spmd
        trace = bool(int(os.environ.get("KERNEL_TRACE", "0")))
        br = run_bass_kernel_spmd(nc, in_maps, list(range(R)), trace=trace)
        results = br.results
        exec_ns = br.exec_time_ns

    n_k, n_e, d_out = meta["n_k"], meta["n_e"], meta["d_out"]
    kn_out = np.concatenate([results[c]["kn_out"] for c in range(R)], axis=0)
    ex_out = np.concatenate([results[c]["ex_out"] for c in range(R)], axis=0)
    assert kn_out.shape == (n_k, d_out) and ex_out.shape == (n_e, d_out)
    return (np.asarray(kn_out, np.float32), np.asarray(ex_out, np.float32)), exec_ns


def kernel(**inputs):
    out, _ = _run(inputs, R=8, sim=False)
    return out


def kernel_timed(**inputs):
    return _run(inputs, R=8, sim=False)


def kernel_sim(R=2, **inputs):
    out, _ = _run(inputs, R=R, sim=True)
    return out


# revision 16
# speedup vs baseline: 1.3088x; 1.1921x over previous
"""Trainium2 Bass kernel: 4x GAT message-passing fusion (gnn_message_passing).

Math reduction used here: the reference GAT has NO nonlinearity between the
edge score e = s_src[src] + s_dst[dst] and the per-dst softmax, so the
s_dst[dst] term (and the segment max m[dst]) cancel inside the softmax:

    alpha_e = exp(s_src[src_e]) / sum_{e'->dst} exp(s_src[src_e'])

Defining per-node g = exp(s_src) and u = g * z, each GAT becomes

    out[dst] = (sum_{e->dst} u[src_e]) / (sum_{e->dst} g[src_e])

i.e. a plain segment-sum of per-node "table" rows [u | g] over incoming
edges, followed by a per-row divide.  No per-edge scalar math at all.

Device plan (8 cores, SPMD):
  Phase A: each core computes its node-shard of the 4 tables
           T = [g*z (d_out) | g | 0-pad]  via bf16 matmuls
           z_aug = h @ [W^T | W^T a_src], g = exp(col d_out).
  AllGather the 4 tables (concatenated per-rank block) -> full table.
  Phase B: edges are sharded by dst; for each 128-dst-node output tile,
           its (padded) incoming edge list is processed in chunks of 128:
           dma_gather 1024 table rows/call, then PE matmul with a host
           precomputed one-hot [128 edge x 128 slot] matrix accumulates
           edge rows into PSUM per dst slot.  Epilogue divides by the
           gathered-g column (guarding isolated nodes) and writes out.
"""

import math
import os
import sys

import numpy as np

for _p in ("/opt/trn_rl_repo", "/opt/trn_rl_repo/concourse"):
    if _p not in sys.path:
        sys.path.insert(0, _p)

import ml_dtypes  # noqa: E402

BF16 = ml_dtypes.bfloat16
P = 128


def _round_up(x, m):
    return (x + m - 1) // m * m


# ----------------------------------------------------------------------------
# Host-side preparation
# ----------------------------------------------------------------------------

def _wrap_idxs(flat, ncalls, idw):
    """[ncalls*idw] -> [P, ncalls*(idw//16)] int16 in dma_gather layout.

    dma_gather reads index i of a call from [partition i%16, col i//16],
    with the 16-partition pattern replicated across the 8 q7 cores.
    """
    idw16 = idw // 16
    a = flat.reshape(ncalls, idw16, 16).transpose(0, 2, 1)  # [ncalls,16,idw16]
    a = np.tile(a, (1, 8, 1))  # [ncalls, 128, idw16]
    return np.ascontiguousarray(a.transpose(1, 0, 2).reshape(P, ncalls * idw16))


def _group_dedup(tile_of_edge, srow_of_edge, slot_of_edge, n_tiles):
    """Group edges by dst tile and deduplicate src rows.

    Within one dst tile, an src row appearing in several edges (to any slots)
    is gathered once; its one-hot row carries per-slot edge counts instead.
    This is exact: out[dst] = sum_e u[src_e] = sum_{unique s} count(s,dst)*u[s].

    Returns per-tile lists: uniq_rows [n_tiles][nu_i] int16 and
    count matrices M [n_tiles][nu_i, P] (uint16)."""
    order = np.argsort(tile_of_edge, kind="stable")
    t_sorted = tile_of_edge[order]
    srow_sorted = srow_of_edge[order]
    slot_sorted = slot_of_edge[order]
    starts = np.searchsorted(t_sorted, np.arange(n_tiles + 1))
    uniq_rows, mats = [], []
    for t in range(n_tiles):
        s, e = starts[t], starts[t + 1]
        rows, inv = np.unique(srow_sorted[s:e], return_inverse=True)
        M = np.zeros((len(rows), P), np.uint16)
        np.add.at(M, (inv, slot_sorted[s:e]), 1)
        uniq_rows.append(rows.astype(np.int16))
        mats.append(M)
    return uniq_rows, mats


def _prepare(inputs, R=8, CPC=8):
    """All host-side preprocessing.  Returns (cfg, in_maps, meta)."""
    kn_emb = np.asarray(inputs["kn_emb"], np.float32)
    exer_emb = np.asarray(inputs["exer_emb"], np.float32)
    n_k, d_in = kn_emb.shape
    n_e = exer_emb.shape[0]
    nb = n_e + n_k
    W = {g: np.asarray(inputs[f"W_{g}"], np.float32) for g in ("dir", "und", "kfe", "efk")}
    A = {g: np.asarray(inputs[f"a_{g}"], np.float32) for g in ("dir", "und", "kfe", "efk")}
    d_out = W["dir"].shape[0]

    assert d_in % P == 0 and d_out % 512 == 0
    assert n_k % (R * P) == 0 and n_e % (R * P) == 0 and nb % (R * P) == 0
    KC = d_in // P
    TW = d_out + P                       # [u (d_out) | g | zeros]
    # matmul N slices: 512-wide (one PSUM bank each) + a single column for g
    # (cols d_out+1.. of the table are zero padding; never computed on PE)
    NSL = [(s, s + 512) for s in range(0, d_out, 512)] + [(d_out, d_out + 1)]
    IDW = CPC * P                        # idxs per dma_gather call

    # --- edges: filter + remap --------------------------------------------
    dir_src = np.asarray(inputs["dir_src"], np.int64)
    dir_dst = np.asarray(inputs["dir_dst"], np.int64)
    und_src = np.asarray(inputs["und_src"], np.int64)
    und_dst = np.asarray(inputs["und_dst"], np.int64)
    kfe_src = np.asarray(inputs["kfe_src"], np.int64)
    kfe_dst = np.asarray(inputs["kfe_dst"], np.int64)
    efk_src = np.asarray(inputs["efk_src"], np.int64)
    efk_dst = np.asarray(inputs["efk_dst"], np.int64)

    mk = kfe_dst >= n_e                  # only kn-dst rows of k_from_e are used
    kfe_s, kfe_d = kfe_src[mk], kfe_dst[mk] - n_e
    me = efk_dst < n_e                   # only exer-dst rows of e_from_k are used
    efk_s, efk_d = efk_src[me], efk_dst[me]

    # kfe src compaction (few kn-dst edges -> few distinct src nodes)
    uniq = np.unique(kfe_s) if len(kfe_s) else np.zeros(1, np.int64)
    KCG = _round_up(len(uniq), R * P)
    KCG = min(KCG, _round_up(nb, R * P))
    assert len(uniq) <= KCG
    kfe_c = np.searchsorted(uniq, kfe_s)  # compact src ids

    # --- table layout (per-rank block) ------------------------------------
    # Two gathered tables:
    #   tab_kn  (one AllGather): per-rank block [dir SH_KN | und SH_KN | kfe SH_KFE]
    #   tab_efk (banded AllGathers): efk rows in NBAND row-bands so the
    #   all-gather of band b can fire as soon as every rank finished band b.
    SH_KN = n_k // R
    SH_KFE = KCG // R
    SH_EFK = nb // R
    KN_BLOCK = 2 * SH_KN + SH_KFE        # per-rank rows in tab_kn
    SH_ROWS = KN_BLOCK + SH_EFK          # per-rank shard rows (bounce layout)
    assert KN_BLOCK * R <= 32767 and SH_EFK * R <= 32767

    EFB = SH_EFK // P                    # efk h-blocks per core
    # NRT shared-output tensors allow a single collective writer, so the efk
    # table is one AllGather (NBAND=1); the kn table's separate early AG still
    # lets phase-B kn work overlap the efk matmuls and the big efk AG.
    NBAND = 1
    BAND_BLOCKS = EFB // NBAND           # h-blocks per band
    BAND_ROWS = BAND_BLOCKS * P          # rows per rank per band

    def row_dir(n):
        return (n // SH_KN) * KN_BLOCK + n % SH_KN

    def row_und(n):
        return (n // SH_KN) * KN_BLOCK + SH_KN + n % SH_KN

    def row_kfe(c):
        return (c // SH_KFE) * KN_BLOCK + 2 * SH_KN + c % SH_KFE

    def row_efk(n):
        rank, local = n // SH_EFK, n % SH_EFK
        band, bl = local // BAND_ROWS, local % BAND_ROWS
        return band * (R * BAND_ROWS) + rank * BAND_ROWS + bl

    # --- per-graph dst sharding -------------------------------------------
    T_KN = SH_KN // P                    # kn dst tiles per core
    T_EFK = n_e // (R * P)               # exer dst tiles per core

    graphs = []  # (name, uniq_rows list, count mats list, n_tiles/core, C)
    for name, (esrc_row, edst, t_per_core, n_nodes_per_core) in {
        "dir": (row_dir(dir_src), dir_dst, T_KN, SH_KN),
        "und": (row_und(und_src), und_dst, T_KN, SH_KN),
        "kfe": (row_kfe(kfe_c), kfe_d, T_KN, SH_KN),
        "efk": (row_efk(efk_s), efk_d, T_EFK, n_e // R),
    }.items():
        core = edst // n_nodes_per_core
        local = edst % n_nodes_per_core
        gtile = core * t_per_core + local // P
        slot = local % P
        n_tiles_total = R * t_per_core
        uniq_rows, mats = _group_dedup(gtile, esrc_row, slot, n_tiles_total)
        nu_max = max((len(u) for u in uniq_rows), default=1)
        C = max(1, _round_up(max(nu_max, 1), P) // P)
        graphs.append((name, uniq_rows, mats, t_per_core, C))

    # --- per-core chunk stream (graph order: dir, und, kfe, efk) ----------
    # chunk_map[i] = (graph_idx, tile_idx_in_core_for_graph, k, C) or None
    # kn-group calls (dir/und/kfe -> gather from tab_kn) come first, then the
    # efk calls (gather from tab_efk); each group starts on a call boundary.
    chunk_map = []
    for gi, (name, _s, _sl, t_per_core, C) in enumerate(graphs):
        for t in range(t_per_core):
            for k in range(C):
                chunk_map.append((gi, t, k, C))
        while len(chunk_map) % CPC:
            chunk_map.append(None)
        if name == "kfe":
            ncalls_kn = len(chunk_map) // CPC
    ncalls = len(chunk_map) // CPC
    nchunks = ncalls * CPC

    idx_streams, oh_streams = [], []
    for c in range(R):
        srcs = np.zeros((nchunks, P), np.int16)
        ohM = np.zeros((nchunks, P, P), np.float32)  # [chunk, src slot, dst slot]
        pos = 0
        for gi, (name, uniq_rows, mats, t_per_core, C) in enumerate(graphs):
            for t in range(t_per_core):
                rows = uniq_rows[c * t_per_core + t]
                M = mats[c * t_per_core + t]
                n = len(rows)
                assert n <= C * P and (M.max() if n else 0) < 256
                base = pos + t * C
                srcs[base:base + C].reshape(-1)[:n] = rows
                ohM[base:base + C].reshape(-1, P)[:n] = M
            pos = _round_up(pos + t_per_core * C, CPC)
        idx_streams.append(_wrap_idxs(srcs.reshape(-1), ncalls, IDW))
        oh = ohM.astype(BF16).reshape(ncalls, CPC, P, P).transpose(0, 2, 1, 3)
        oh_streams.append(np.ascontiguousarray(oh.reshape(ncalls, P, CPC * P)))

    # --- phase A: h blocks + weights --------------------------------------
    kn_bf = kn_emb.astype(BF16)
    ek_bf = np.concatenate([exer_emb.astype(BF16), kn_bf], axis=0)
    KFB = SH_KFE // P
    EFB = SH_EFK // P
    NBLK = T_KN + KFB + EFB

    hT_cores = []
    for c in range(R):
        rows = np.zeros((NBLK * P, d_in), BF16)
        rows[:SH_KN] = kn_bf[c * SH_KN:(c + 1) * SH_KN]
        lo, hi = c * SH_KFE, (c + 1) * SH_KFE
        take = uniq[lo:min(hi, len(uniq))]
        rows[SH_KN:SH_KN + len(take)] = ek_bf[take]
        rows[SH_KN + SH_KFE:] = ek_bf[c * SH_EFK:(c + 1) * SH_EFK]
        hT = rows.reshape(NBLK, P, KC, P).transpose(0, 3, 2, 1)  # [b, p, kc, m]
        hT_cores.append(np.ascontiguousarray(hT.reshape(NBLK, P, KC * P)))

    Wt = np.zeros((4, P, KC * TW), BF16)
    for gi, g in enumerate(("dir", "und", "kfe", "efk")):
        waug = np.zeros((d_in, TW), np.float32)
        waug[:, :d_out] = W[g].T
        waug[:, d_out] = W[g].T @ A[g][:d_out]
        Wt[gi] = waug.astype(BF16).reshape(KC, P, TW).transpose(1, 0, 2).reshape(P, KC * TW)

    # phase A m-tile list: (h_block, graph_id, shard_row0)
    mtiles = []
    for t in range(T_KN):
        mtiles.append((t, 0, t * P))
        mtiles.append((t, 1, SH_KN + t * P))
    for j in range(KFB):
        mtiles.append((T_KN + j, 2, 2 * SH_KN + j * P))
    for j in range(EFB):
        mtiles.append((T_KN + KFB + j, 3, KN_BLOCK + j * P))

    cfg = dict(
        R=R, KC=KC, TW=TW, NSL=NSL, CPC=CPC, IDW=IDW, d_out=d_out,
        SH_ROWS=SH_ROWS, NBLK=NBLK, ncalls=ncalls, nchunks=nchunks,
        ncalls_kn=ncalls_kn, KN_BLOCK=KN_BLOCK, NBAND=NBAND,
        BAND_ROWS=BAND_ROWS, SH_EFK=SH_EFK,
        mtiles=mtiles, chunk_map=chunk_map, T_KN=T_KN, T_EFK=T_EFK,
        SH_KN=SH_KN, n_kn_graphs=3,
    )
    in_maps = [
        {"hT": hT_cores[c], "Wt": Wt, "idx": idx_streams[c], "oh": oh_streams[c]}
        for c in range(R)
    ]
    meta = dict(n_k=n_k, n_e=n_e, d_out=d_out)
    return cfg, in_maps, meta


# ----------------------------------------------------------------------------
# Device program
# ----------------------------------------------------------------------------

def _build(cfg, debug=False, asserts=False):
    import concourse.bacc as bacc
    import concourse.mybir as mybir
    import concourse.tile as tile
    from concourse.library_config import mlp

    dt = mybir.dt
    AOT = mybir.AluOpType
    R, KC, TW, CPC, IDW = cfg["R"], cfg["KC"], cfg["TW"], cfg["CPC"], cfg["IDW"]
    NSL, d_out = cfg["NSL"], cfg["d_out"]
    SH_ROWS, NBLK, ncalls = cfg["SH_ROWS"], cfg["NBLK"], cfg["ncalls"]
    T_KN, T_EFK, SH_KN = cfg["T_KN"], cfg["T_EFK"], cfg["SH_KN"]
    IDW16 = IDW // 16

    ncalls_kn = cfg["ncalls_kn"]
    KN_BLOCK, NBAND = cfg["KN_BLOCK"], cfg["NBAND"]
    BAND_ROWS, SH_EFK = cfg["BAND_ROWS"], cfg["SH_EFK"]

    nc = bacc.Bacc("TRN2", target_bir_lowering=False, debug=debug,
                   enable_asserts=asserts, num_devices=R)

    HT = nc.dram_tensor("hT", [NBLK, P, KC * P], dt.bfloat16, kind="ExternalInput")
    WT = nc.dram_tensor("Wt", [4, P, KC * TW], dt.bfloat16, kind="ExternalInput")
    IDX = nc.dram_tensor("idx", [P, ncalls * IDW16], dt.int16, kind="ExternalInput")
    OH = nc.dram_tensor("oh", [ncalls, P, CPC * P], dt.bfloat16, kind="ExternalInput")
    KN_OUT = nc.dram_tensor("kn_out", [SH_KN, d_out], dt.float32, kind="ExternalOutput")
    EX_OUT = nc.dram_tensor("ex_out", [T_EFK * P, d_out], dt.float32, kind="ExternalOutput")

    with tile.TileContext(nc) as tc:
        nc.gpsimd.load_library(mlp)
        with tc.tile_pool(name="dram", bufs=1, space="DRAM") as dramp:
            shard = dramp.tile([SH_ROWS, TW], dt.bfloat16, name="shard_tab")
            tab_kn = dramp.tile([KN_BLOCK * R, TW], dt.bfloat16,
                                addr_space="Shared", name="tab_kn")
            tab_efk = dramp.tile([SH_EFK * R, TW], dt.bfloat16,
                                 addr_space="Shared", name="tab_efk")
            rg = [list(range(R))]

            def ag(shard_r0, nrows, out_tab, out_r0):
                nc.gpsimd.collective_compute(
                    "AllGather", AOT.bypass, replica_groups=rg,
                    ins=[shard[shard_r0:shard_r0 + nrows, :]],
                    outs=[out_tab[out_r0:out_r0 + nrows * R, :]],
                )

            # ---------------- Phase A: build table shard ----------------
            with (
                tc.tile_pool(name="wp", bufs=2) as wp,
                tc.tile_pool(name="hp", bufs=3) as hp,
                tc.tile_pool(name="psA", bufs=2, space="PSUM") as psA,
                tc.tile_pool(name="tabp", bufs=3) as tabp,
                tc.tile_pool(name="gsp", bufs=4) as gsp,
            ):
                n_kn_mtiles = sum(1 for (_b, g, _r) in cfg["mtiles"] if g < 3)
                efk_done = 0
                w_sb = {}
                h_sb, cur_blk = None, None
                for mi, (b, g, r0) in enumerate(cfg["mtiles"]):
                    if g not in w_sb:
                        # pool bufs=2 keeps the two live graphs' W resident;
                        # older allocations' slots are recycled by the pool
                        w = wp.tile([P, KC * TW], dt.bfloat16, tag="w", name=f"w{g}")
                        nc.sync.dma_start(out=w[:], in_=WT.ap()[g])
                        w_sb[g] = w
                    if b != cur_blk:
                        h_sb = hp.tile([P, KC * P], dt.bfloat16, tag="h", name=f"h{b}")
                        nc.sync.dma_start(out=h_sb[:], in_=HT.ap()[b])
                        cur_blk = b
                    w = w_sb[g]
                    ps = psA.tile([P, TW], dt.float32, tag="psA", name=f"psA{b}_{g}")
                    for (s0, s1) in NSL:
                        for kc in range(KC):
                            nc.tensor.matmul(
                                out=ps[:, s0:s1],
                                lhsT=h_sb[:, kc * P:(kc + 1) * P],
                                rhs=w[:, kc * TW + s0:kc * TW + s1],
                                start=(kc == 0), stop=(kc == KC - 1),
                            )
                    gv = gsp.tile([P, 1], dt.float32, tag="gv", name=f"gv{b}_{g}")
                    nc.scalar.activation(gv[:, :1], ps[:, d_out:d_out + 1],
                                         mybir.ActivationFunctionType.Exp)
                    tab = tabp.tile([P, TW], dt.bfloat16, tag="tab", name=f"tab{b}_{g}")
                    nc.vector.tensor_scalar(tab[:, 0:d_out], ps[:, 0:d_out],
                                            gv[:, :1], None, AOT.mult)
                    nc.vector.tensor_copy(out=tab[:, d_out:d_out + 1], in_=gv[:, :1])
                    nc.vector.memset(tab[:, d_out + 1:TW], 0.0)
                    nc.sync.dma_start(out=shard[r0:r0 + P, :], in_=tab[:])
                    # fire the all-gathers as soon as their band is complete
                    if mi == n_kn_mtiles - 1:
                        ag(0, KN_BLOCK, tab_kn, 0)
                    if g == 3:
                        efk_done += 1
                        if efk_done % (BAND_ROWS // P) == 0:
                            band = efk_done // (BAND_ROWS // P) - 1
                            ag(KN_BLOCK + band * BAND_ROWS, BAND_ROWS,
                               tab_efk, band * BAND_ROWS * R)

            # ---------------- Phase B: gather + aggregate ----------------
            with (
                tc.tile_pool(name="idxp", bufs=1) as idxp,
                tc.tile_pool(name="gp", bufs=3) as gp,
                tc.tile_pool(name="ohp", bufs=3) as ohp,
                tc.tile_pool(name="psB", bufs=2, space="PSUM") as psB,
                tc.tile_pool(name="accp", bufs=max(1, T_KN)) as accp,
                tc.tile_pool(name="outp", bufs=3) as outp,
                tc.tile_pool(name="eps", bufs=8) as eps,
            ):
                idx_sb = idxp.tile([P, ncalls * IDW16], dt.int16, name="idx_sb")
                nc.sync.dma_start(out=idx_sb[:], in_=IDX.ap()[:, :])

                acc = {}     # kn tile -> acc sbuf tile
                cur_ps = {}  # (graph,tile) currently accumulating
                n_kn_graphs = cfg["n_kn_graphs"]

                def epilogue(gi, t, ps):
                    den = eps.tile([P, 1], dt.float32, tag="den", name=f"den{gi}_{t}")
                    z0 = eps.tile([P, 1], dt.float32, tag="z0", name=f"z0{gi}_{t}")
                    nc.vector.tensor_scalar(z0[:, :1], ps[:, d_out:d_out + 1],
                                            0.0, None, AOT.is_equal)
                    nc.vector.tensor_tensor(out=den[:, :1], in0=ps[:, d_out:d_out + 1],
                                            in1=z0[:, :1], op=AOT.add)
                    rec = eps.tile([P, 1], dt.float32, tag="rec", name=f"rec{gi}_{t}")
                    nc.vector.reciprocal(rec[:, :1], den[:, :1])
                    if gi == 0:
                        a = accp.tile([P, d_out], dt.float32, tag="acc", name=f"acc{t}")
                        acc[t] = a
                        nc.vector.tensor_scalar(a[:], ps[:, 0:d_out], rec[:, :1],
                                                None, AOT.mult)
                    elif gi < n_kn_graphs:
                        tmp = outp.tile([P, d_out], dt.float32, tag="o", name=f"tmp{gi}_{t}")
                        nc.vector.tensor_scalar(tmp[:], ps[:, 0:d_out], rec[:, :1],
                                                None, AOT.mult)
                        nc.vector.tensor_tensor(out=acc[t][:], in0=acc[t][:],
                                                in1=tmp[:], op=AOT.add)
                        if gi == n_kn_graphs - 1:
                            nc.sync.dma_start(out=KN_OUT.ap()[t * P:(t + 1) * P, :],
                                              in_=acc[t][:])
                    else:
                        o = outp.tile([P, d_out], dt.float32, tag="o", name=f"o{t}")
                        nc.vector.tensor_scalar(o[:], ps[:, 0:d_out], rec[:, :1],
                                                None, AOT.mult)
                        nc.sync.dma_start(out=EX_OUT.ap()[t * P:(t + 1) * P, :], in_=o[:])

                for call in range(ncalls):
                    src_tab = tab_kn if call < ncalls_kn else tab_efk
                    gt = gp.tile([P, CPC, TW], dt.bfloat16, tag="gt", name=f"gt{call}")
                    nc.gpsimd.dma_gather(
                        gt[:], src_tab[:, :],
                        idx_sb[:, call * IDW16:(call + 1) * IDW16],
                        IDW, IDW, TW,
                    )
                    oh_sb = ohp.tile([P, CPC * P], dt.bfloat16, tag="oh", name=f"oh{call}")
                    nc.sync.dma_start(out=oh_sb[:], in_=OH.ap()[call])
                    for c in range(CPC):
                        cm = cfg["chunk_map"][call * CPC + c]
                        if cm is None:
                            continue
                        gi, t, k, C = cm
                        if k == 0:
                            ps = psB.tile([P, TW], dt.float32, tag="psB",
                                          name=f"psB{gi}_{t}")
                            cur_ps[(gi, t)] = ps
                        ps = cur_ps[(gi, t)]
                        for (s0, s1) in NSL:
                            nc.tensor.matmul(
                                out=ps[:, s0:s1],
                                lhsT=oh_sb[:, c * P:(c + 1) * P],
                                rhs=gt[:, c, s0:s1],
                                start=(k == 0), stop=(k == C - 1),
                            )
                        if k == C - 1:
                            epilogue(gi, t, ps)

    nc.compile()
    return nc


# ----------------------------------------------------------------------------
# Entry point
# ----------------------------------------------------------------------------

_CACHE = {}


def _run(inputs, R=8, sim=False):
    cfg, in_maps, meta = _prepare(inputs, R=R)

    key = (R, cfg["ncalls"], cfg["ncalls_kn"], cfg["NBAND"], cfg["NBLK"],
           cfg["SH_ROWS"], cfg["TW"], cfg["KC"],
           tuple(x if x is None else x[:3] for x in cfg["chunk_map"]), sim)
    if key in _CACHE:
        nc = _CACHE[key]
    else:
        nc = _build(cfg, debug=sim, asserts=sim)
        _CACHE[key] = nc

    if sim:
        from concourse.bass_interp import MultiCoreSim
        msim = MultiCoreSim(nc, num_cores=R)
        for c in range(R):
            for k, v in in_maps[c].items():
                msim.cores[c].tensor(k)[:] = v
        msim.simulate(check_with_hw=False)
        results = [
            {"kn_out": np.array(msim.cores[c].tensor("kn_out")),
             "ex_out": np.array(msim.cores[c].tensor("ex_out"))}
            for c in range(R)
        ]
        exec_ns = None
    else:
        from concourse.bass_utils import run_bass_kernel_spmd
        trace = bool(int(os.environ.get("KERNEL_TRACE", "0")))
        br = run_bass_kernel_spmd(nc, in_maps, list(range(R)), trace=trace)
        results = br.results
        exec_ns = br.exec_time_ns

    n_k, n_e, d_out = meta["n_k"], meta["n_e"], meta["d_out"]
    kn_out = np.concatenate([results[c]["kn_out"] for c in range(R)], axis=0)
    ex_out = np.concatenate([results[c]["ex_out"] for c in range(R)], axis=0)
    assert kn_out.shape == (n_k, d_out) and ex_out.shape == (n_e, d_out)
    return (np.asarray(kn_out, np.float32), np.asarray(ex_out, np.float32)), exec_ns


def kernel(**inputs):
    out, _ = _run(inputs, R=8, sim=False)
    return out


def kernel_timed(**inputs):
    return _run(inputs, R=8, sim=False)


def kernel_sim(R=2, **inputs):
    out, _ = _run(inputs, R=R, sim=True)
    return out


# revision 27
# speedup vs baseline: 1.3155x; 1.0051x over previous
"""Trainium2 Bass kernel: 4x GAT message-passing fusion (gnn_message_passing).

Math reduction used here: the reference GAT has NO nonlinearity between the
edge score e = s_src[src] + s_dst[dst] and the per-dst softmax, so the
s_dst[dst] term (and the segment max m[dst]) cancel inside the softmax:

    alpha_e = exp(s_src[src_e]) / sum_{e'->dst} exp(s_src[src_e'])

Defining per-node g = exp(s_src) and u = g * z, each GAT becomes

    out[dst] = (sum_{e->dst} u[src_e]) / (sum_{e->dst} g[src_e])

i.e. a plain segment-sum of per-node "table" rows [u | g] over incoming
edges, followed by a per-row divide.  No per-edge scalar math at all.

Device plan (8 cores, SPMD):
  Phase A: each core computes its node-shard of the 4 tables
           T = [g*z (d_out) | g | 0-pad]  via bf16 matmuls
           z_aug = h @ [W^T | W^T a_src], g = exp(col d_out).
  AllGather the 4 tables (concatenated per-rank block) -> full table.
  Phase B: edges are sharded by dst; for each 128-dst-node output tile,
           its (padded) incoming edge list is processed in chunks of 128:
           dma_gather 1024 table rows/call, then PE matmul with a host
           precomputed one-hot [128 edge x 128 slot] matrix accumulates
           edge rows into PSUM per dst slot.  Epilogue divides by the
           gathered-g column (guarding isolated nodes) and writes out.
"""

import math
import os
import sys

import numpy as np

for _p in ("/opt/trn_rl_repo", "/opt/trn_rl_repo/concourse"):
    if _p not in sys.path:
        sys.path.insert(0, _p)

import ml_dtypes  # noqa: E402

BF16 = ml_dtypes.bfloat16
P = 128


def _round_up(x, m):
    return (x + m - 1) // m * m


# ----------------------------------------------------------------------------
# Host-side preparation
# ----------------------------------------------------------------------------

def _wrap_idxs(flat, ncalls, idw):
    """[ncalls*idw] -> [P, ncalls*(idw//16)] int16 in dma_gather layout.

    dma_gather reads index i of a call from [partition i%16, col i//16],
    with the 16-partition pattern replicated across the 8 q7 cores.
    """
    idw16 = idw // 16
    a = flat.reshape(ncalls, idw16, 16).transpose(0, 2, 1)  # [ncalls,16,idw16]
    a = np.tile(a, (1, 8, 1))  # [ncalls, 128, idw16]
    return np.ascontiguousarray(a.transpose(1, 0, 2).reshape(P, ncalls * idw16))


def _group_dedup(tile_of_edge, srow_of_edge, slot_of_edge, n_tiles):
    """Group edges by dst tile and deduplicate src rows.

    Within one dst tile, an src row appearing in several edges (to any slots)
    is gathered once; its one-hot row carries per-slot edge counts instead.
    This is exact: out[dst] = sum_e u[src_e] = sum_{unique s} count(s,dst)*u[s].

    Returns per-tile lists: uniq_rows [n_tiles][nu_i] int16 and
    count matrices M [n_tiles][nu_i, P] (uint16)."""
    order = np.argsort(tile_of_edge, kind="stable")
    t_sorted = tile_of_edge[order]
    srow_sorted = srow_of_edge[order]
    slot_sorted = slot_of_edge[order]
    starts = np.searchsorted(t_sorted, np.arange(n_tiles + 1))
    uniq_rows, mats = [], []
    for t in range(n_tiles):
        s, e = starts[t], starts[t + 1]
        rows, inv = np.unique(srow_sorted[s:e], return_inverse=True)
        M = np.zeros((len(rows), P), np.uint16)
        np.add.at(M, (inv, slot_sorted[s:e]), 1)
        uniq_rows.append(rows.astype(np.int16))
        mats.append(M)
    return uniq_rows, mats


def _prepare(inputs, R=8, CPC=8):
    """All host-side preprocessing.  Returns (cfg, in_maps, meta)."""
    kn_emb = np.asarray(inputs["kn_emb"], np.float32)
    exer_emb = np.asarray(inputs["exer_emb"], np.float32)
    n_k, d_in = kn_emb.shape
    n_e = exer_emb.shape[0]
    nb = n_e + n_k
    W = {g: np.asarray(inputs[f"W_{g}"], np.float32) for g in ("dir", "und", "kfe", "efk")}
    A = {g: np.asarray(inputs[f"a_{g}"], np.float32) for g in ("dir", "und", "kfe", "efk")}
    d_out = W["dir"].shape[0]

    assert d_in % P == 0 and d_out % 512 == 0
    assert n_k % (R * P) == 0 and n_e % (R * P) == 0 and nb % (R * P) == 0
    KC = d_in // P
    TW = d_out + P                       # [u (d_out) | g | zeros]
    # matmul N slices: 512-wide (one PSUM bank each) + a single column for g
    # (cols d_out+1.. of the table are zero padding; never computed on PE)
    NSL = [(s, s + 512) for s in range(0, d_out, 512)] + [(d_out, d_out + 1)]
    IDW = CPC * P                        # idxs per dma_gather call

    # --- edges: filter + remap --------------------------------------------
    dir_src = np.asarray(inputs["dir_src"], np.int64)
    dir_dst = np.asarray(inputs["dir_dst"], np.int64)
    und_src = np.asarray(inputs["und_src"], np.int64)
    und_dst = np.asarray(inputs["und_dst"], np.int64)
    kfe_src = np.asarray(inputs["kfe_src"], np.int64)
    kfe_dst = np.asarray(inputs["kfe_dst"], np.int64)
    efk_src = np.asarray(inputs["efk_src"], np.int64)
    efk_dst = np.asarray(inputs["efk_dst"], np.int64)

    mk = kfe_dst >= n_e                  # only kn-dst rows of k_from_e are used
    kfe_s, kfe_d = kfe_src[mk], kfe_dst[mk] - n_e
    me = efk_dst < n_e                   # only exer-dst rows of e_from_k are used
    efk_s, efk_d = efk_src[me], efk_dst[me]

    # kfe src compaction (few kn-dst edges -> few distinct src nodes)
    uniq = np.unique(kfe_s) if len(kfe_s) else np.zeros(1, np.int64)
    KCG = _round_up(len(uniq), R * P)
    KCG = min(KCG, _round_up(nb, R * P))
    assert len(uniq) <= KCG
    kfe_c = np.searchsorted(uniq, kfe_s)  # compact src ids

    # --- table layout (per-rank block) ------------------------------------
    # Two gathered tables:
    #   tab_kn  (one AllGather): per-rank block [dir SH_KN | und SH_KN | kfe SH_KFE]
    #   tab_efk (banded AllGathers): efk rows in NBAND row-bands so the
    #   all-gather of band b can fire as soon as every rank finished band b.
    SH_KN = n_k // R
    SH_KFE = KCG // R
    SH_EFK = nb // R
    KN_BLOCK = 2 * SH_KN + SH_KFE        # per-rank rows in tab_kn
    SH_ROWS = KN_BLOCK + SH_EFK          # per-rank shard rows (bounce layout)
    assert KN_BLOCK * R <= 32767 and SH_EFK * R <= 32767

    EFB = SH_EFK // P                    # efk h-blocks per core
    # The efk table is all-gathered in bands, each band a separate Shared
    # tensor (NRT allows one collective writer per shared tensor).  Front
    # bands are big (their AG overlaps the remaining efk matmuls); the last
    # band is small so the exposed AG tail is short.
    if EFB >= 7:
        b3 = max(EFB // 7, 1)
        b1 = (EFB - b3 + 1) // 2
        BANDS = [b1, EFB - b3 - b1, b3]
    elif EFB >= 3:
        BANDS = [EFB - 2, 1, 1]
    else:
        BANDS = [EFB]
    BANDS = [b for b in BANDS if b > 0]
    NBAND = len(BANDS)
    BAND_ROWS = [b * P for b in BANDS]   # rows per rank per band
    BAND_START = np.concatenate([[0], np.cumsum(BAND_ROWS)]).astype(np.int64)

    def row_dir(n):
        return (n // SH_KN) * KN_BLOCK + n % SH_KN

    def row_und(n):
        return (n // SH_KN) * KN_BLOCK + SH_KN + n % SH_KN

    def row_kfe(c):
        return (c // SH_KFE) * KN_BLOCK + 2 * SH_KN + c % SH_KFE

    def band_of_efk(n):
        local = n % SH_EFK
        return np.searchsorted(BAND_START, local, side="right") - 1

    def row_efk_in_band(n, band):
        rank, local = n // SH_EFK, n % SH_EFK
        return rank * BAND_ROWS[band] + (local - BAND_START[band])

    # --- per-graph dst sharding -------------------------------------------
    T_KN = SH_KN // P                    # kn dst tiles per core
    T_EFK = n_e // (R * P)               # exer dst tiles per core

    # streams: (name, table_id, out_gi, uniq_rows, mats, t_per_core, C)
    # table_id: 0 -> tab_kn, 1+b -> tab_efk band b.  out_gi routes epilogues.
    efk_band = band_of_efk(efk_s)
    stream_defs = [
        ("dir", 0, 0, row_dir(dir_src), dir_dst, T_KN, SH_KN),
        ("und", 0, 1, row_und(und_src), und_dst, T_KN, SH_KN),
        ("kfe", 0, 2, row_kfe(kfe_c), kfe_d, T_KN, SH_KN),
    ] + [
        ("efk%d" % b, 1 + b, 3,
         row_efk_in_band(efk_s[efk_band == b], b),
         efk_d[efk_band == b], T_EFK, n_e // R)
        for b in range(NBAND)
    ]
    streams = []
    for (name, tid, ogi, esrc_row, edst, t_per_core, n_nodes_per_core) in stream_defs:
        core = edst // n_nodes_per_core
        local = edst % n_nodes_per_core
        gtile = core * t_per_core + local // P
        slot = local % P
        n_tiles_total = R * t_per_core
        uniq_rows, mats = _group_dedup(gtile, esrc_row, slot, n_tiles_total)
        nu_max = max((len(u) for u in uniq_rows), default=1)
        C = max(1, _round_up(max(nu_max, 1), P) // P)
        streams.append((name, tid, ogi, uniq_rows, mats, t_per_core, C))

    # call layout: per stream, chunks are tile-major (j = t*C + k), padded to
    # a CPC boundary; global call id follows stream order.
    start_call = []
    ncalls = 0
    for (name, tid, ogi, _u, _m, t_per_core, C) in streams:
        start_call.append(ncalls)
        ncalls += _round_up(t_per_core * C, CPC) // CPC
    nchunks = ncalls * CPC
    call_tab = np.zeros(ncalls, np.int64)
    for si, (name, tid, *_r) in enumerate(streams):
        end = start_call[si + 1] if si + 1 < len(streams) else ncalls
        call_tab[start_call[si]:end] = tid

    # consumption schedule: kn streams graph-major, then efk tile-major
    # across bands (one PSUM accumulation per dst tile spanning all bands).
    # events: ("g", call) gather+onehot load; ("c", call, slot, out_gi, t,
    # start, stop) matmul chunk (epilogue when stop on last kn graph / efk).
    sched = []
    seen_calls = set()

    def chunk_ev(si, t, k, start, stop):
        name, tid, ogi, _u, _m, t_per_core, C = streams[si]
        j = t * C + k
        call = start_call[si] + j // CPC
        if call not in seen_calls:
            sched.append(("g", int(call)))
            seen_calls.add(call)
        sched.append(("c", int(call), int(j % CPC), ogi, int(t), start, stop))

    for si in range(3):
        C = streams[si][6]
        for t in range(streams[si][5]):
            for k in range(C):
                chunk_ev(si, t, k, k == 0, k == C - 1)
    efk_sis = list(range(3, len(streams)))
    for t in range(T_EFK):
        for bi, si in enumerate(efk_sis):
            C = streams[si][6]
            for k in range(C):
                chunk_ev(si, t, k,
                         bi == 0 and k == 0,
                         bi == len(efk_sis) - 1 and k == C - 1)

    idx_streams, oh_streams = [], []
    for c in range(R):
        srcs = np.zeros((nchunks, P), np.int16)
        ohM = np.zeros((nchunks, P, P), np.float32)  # [chunk, src slot, dst slot]
        for si, (name, tid, ogi, uniq_rows, mats, t_per_core, C) in enumerate(streams):
            pos = start_call[si] * CPC
            for t in range(t_per_core):
                rows = uniq_rows[c * t_per_core + t]
                M = mats[c * t_per_core + t]
                n = len(rows)
                assert n <= C * P and (M.max() if n else 0) < 256
                base = pos + t * C
                srcs[base:base + C].reshape(-1)[:n] = rows
                ohM[base:base + C].reshape(-1, P)[:n] = M
        idx_streams.append(_wrap_idxs(srcs.reshape(-1), ncalls, IDW))
        oh = ohM.astype(BF16).reshape(ncalls, CPC, P, P).transpose(0, 2, 1, 3)
        oh_streams.append(np.ascontiguousarray(oh.reshape(ncalls, P, CPC * P)))

    # --- phase A: h blocks + weights --------------------------------------
    kn_bf = kn_emb.astype(BF16)
    ek_bf = np.concatenate([exer_emb.astype(BF16), kn_bf], axis=0)
    KFB = SH_KFE // P
    EFB = SH_EFK // P
    NBLK = T_KN + KFB + EFB

    hT_cores = []
    for c in range(R):
        rows = np.zeros((NBLK * P, d_in), BF16)
        rows[:SH_KN] = kn_bf[c * SH_KN:(c + 1) * SH_KN]
        lo, hi = c * SH_KFE, (c + 1) * SH_KFE
        take = uniq[lo:min(hi, len(uniq))]
        rows[SH_KN:SH_KN + len(take)] = ek_bf[take]
        rows[SH_KN + SH_KFE:] = ek_bf[c * SH_EFK:(c + 1) * SH_EFK]
        hT = rows.reshape(NBLK, P, KC, P).transpose(0, 3, 2, 1)  # [b, p, kc, m]
        hT_cores.append(np.ascontiguousarray(hT.reshape(NBLK, P, KC * P)))

    Wt = np.zeros((4, P, KC * TW), BF16)
    for gi, g in enumerate(("dir", "und", "kfe", "efk")):
        waug = np.zeros((d_in, TW), np.float32)
        waug[:, :d_out] = W[g].T
        waug[:, d_out] = W[g].T @ A[g][:d_out]
        Wt[gi] = waug.astype(BF16).reshape(KC, P, TW).transpose(1, 0, 2).reshape(P, KC * TW)

    # phase A m-tile list: (h_block, graph_id, shard_row0)
    mtiles = []
    for t in range(T_KN):
        mtiles.append((t, 0, t * P))
        mtiles.append((t, 1, SH_KN + t * P))
    for j in range(KFB):
        mtiles.append((T_KN + j, 2, 2 * SH_KN + j * P))
    for j in range(EFB):
        mtiles.append((T_KN + KFB + j, 3, KN_BLOCK + j * P))

    cfg = dict(
        R=R, KC=KC, TW=TW, NSL=NSL, CPC=CPC, IDW=IDW, d_out=d_out,
        SH_ROWS=SH_ROWS, NBLK=NBLK, ncalls=ncalls, nchunks=nchunks,
        KN_BLOCK=KN_BLOCK, NBAND=NBAND, BANDS=BANDS,
        BAND_ROWS=BAND_ROWS, SH_EFK=SH_EFK,
        mtiles=mtiles, sched=sched, call_tab=[int(x) for x in call_tab],
        T_KN=T_KN, T_EFK=T_EFK,
        SH_KN=SH_KN, n_kn_graphs=3,
    )
    in_maps = [
        {"hT": hT_cores[c], "Wt": Wt, "idx": idx_streams[c], "oh": oh_streams[c]}
        for c in range(R)
    ]
    meta = dict(n_k=n_k, n_e=n_e, d_out=d_out)
    return cfg, in_maps, meta


# ----------------------------------------------------------------------------
# Device program
# ----------------------------------------------------------------------------

def _build(cfg, debug=False, asserts=False):
    import concourse.bacc as bacc
    import concourse.mybir as mybir
    import concourse.tile as tile
    from concourse.library_config import mlp

    dt = mybir.dt
    AOT = mybir.AluOpType
    R, KC, TW, CPC, IDW = cfg["R"], cfg["KC"], cfg["TW"], cfg["CPC"], cfg["IDW"]
    NSL, d_out = cfg["NSL"], cfg["d_out"]
    SH_ROWS, NBLK, ncalls = cfg["SH_ROWS"], cfg["NBLK"], cfg["ncalls"]
    T_KN, T_EFK, SH_KN = cfg["T_KN"], cfg["T_EFK"], cfg["SH_KN"]
    IDW16 = IDW // 16

    KN_BLOCK, NBAND = cfg["KN_BLOCK"], cfg["NBAND"]
    BAND_ROWS, SH_EFK = cfg["BAND_ROWS"], cfg["SH_EFK"]
    BANDS, call_tab = cfg["BANDS"], cfg["call_tab"]

    nc = bacc.Bacc("TRN2", target_bir_lowering=False, debug=debug,
                   enable_asserts=asserts, num_devices=R)

    HT = nc.dram_tensor("hT", [NBLK, P, KC * P], dt.bfloat16, kind="ExternalInput")
    WT = nc.dram_tensor("Wt", [4, P, KC * TW], dt.bfloat16, kind="ExternalInput")
    IDX = nc.dram_tensor("idx", [P, ncalls * IDW16], dt.int16, kind="ExternalInput")
    OH = nc.dram_tensor("oh", [ncalls, P, CPC * P], dt.bfloat16, kind="ExternalInput")
    KN_OUT = nc.dram_tensor("kn_out", [SH_KN, d_out], dt.float32, kind="ExternalOutput")
    EX_OUT = nc.dram_tensor("ex_out", [T_EFK * P, d_out], dt.float32, kind="ExternalOutput")

    with tile.TileContext(nc) as tc:
        nc.gpsimd.load_library(mlp)
        with tc.tile_pool(name="dram", bufs=1, space="DRAM") as dramp:
            shard = dramp.tile([SH_ROWS, TW], dt.bfloat16, name="shard_tab")
            tab_kn = dramp.tile([KN_BLOCK * R, TW], dt.bfloat16,
                                addr_space="Shared", name="tab_kn")
            tab_efk = [
                dramp.tile([BAND_ROWS[b] * R, TW], dt.bfloat16,
                           addr_space="Shared", name=f"tab_efk{b}")
                for b in range(NBAND)
            ]
            tables = [tab_kn] + tab_efk
            rg = [list(range(R))]

            def ag(shard_r0, nrows, out_tab):
                nc.gpsimd.collective_compute(
                    "AllGather", AOT.bypass, replica_groups=rg,
                    ins=[shard[shard_r0:shard_r0 + nrows, :]],
                    outs=[out_tab[:, :]],
                )

            # ---------------- Phase A: build table shard ----------------
            with (
                tc.tile_pool(name="wp", bufs=2) as wp,
                tc.tile_pool(name="hp", bufs=3) as hp,
                tc.tile_pool(name="psA", bufs=2, space="PSUM") as psA,
                tc.tile_pool(name="tabp", bufs=3) as tabp,
                tc.tile_pool(name="gsp", bufs=4) as gsp,
            ):
                n_kn_mtiles = sum(1 for (_b, g, _r) in cfg["mtiles"] if g < 3)
                band_ends = list(np.cumsum(BANDS))
                band_row_start = [0] + [int(x) * P for x in np.cumsum(BANDS)][:-1]
                efk_done = 0
                w_sb = {}
                h_sb, cur_blk = None, None
                for mi, (b, g, r0) in enumerate(cfg["mtiles"]):
                    if g not in w_sb:
                        # pool bufs=2 keeps the two live graphs' W resident;
                        # older allocations' slots are recycled by the pool
                        w = wp.tile([P, KC * TW], dt.bfloat16, tag="w", name=f"w{g}")
                        nc.sync.dma_start(out=w[:], in_=WT.ap()[g])
                        w_sb[g] = w
                    if b != cur_blk:
                        h_sb = hp.tile([P, KC * P], dt.bfloat16, tag="h", name=f"h{b}")
                        nc.sync.dma_start(out=h_sb[:], in_=HT.ap()[b])
                        cur_blk = b
                    w = w_sb[g]
                    ps = psA.tile([P, TW], dt.float32, tag="psA", name=f"psA{b}_{g}")
                    for (s0, s1) in NSL:
                        for kc in range(KC):
                            nc.tensor.matmul(
                                out=ps[:, s0:s1],
                                lhsT=h_sb[:, kc * P:(kc + 1) * P],
                                rhs=w[:, kc * TW + s0:kc * TW + s1],
                                start=(kc == 0), stop=(kc == KC - 1),
                            )
                    gv = gsp.tile([P, 1], dt.float32, tag="gv", name=f"gv{b}_{g}")
                    nc.scalar.activation(gv[:, :1], ps[:, d_out:d_out + 1],
                                         mybir.ActivationFunctionType.Exp)
                    tab = tabp.tile([P, TW], dt.bfloat16, tag="tab", name=f"tab{b}_{g}")
                    nc.vector.tensor_scalar(tab[:, 0:d_out], ps[:, 0:d_out],
                                            gv[:, :1], None, AOT.mult)
                    nc.vector.tensor_copy(out=tab[:, d_out:d_out + 1], in_=gv[:, :1])
                    nc.vector.memset(tab[:, d_out + 1:TW], 0.0)
                    nc.sync.dma_start(out=shard[r0:r0 + P, :], in_=tab[:])
                    # fire each all-gather as soon as its band is complete
                    if mi == n_kn_mtiles - 1:
                        ag(0, KN_BLOCK, tab_kn)
                    if g == 3:
                        efk_done += 1
                        if efk_done in band_ends:
                            band = band_ends.index(efk_done)
                            ag(KN_BLOCK + band_row_start[band],
                               BAND_ROWS[band], tab_efk[band])

            # ---------------- Phase B: gather + aggregate ----------------
            with (
                tc.tile_pool(name="idxp", bufs=1) as idxp,
                tc.tile_pool(name="gp", bufs=6) as gp,
                tc.tile_pool(name="ohp", bufs=6) as ohp,
                tc.tile_pool(name="psB", bufs=2, space="PSUM") as psB,
                tc.tile_pool(name="accp", bufs=max(1, T_KN)) as accp,
                tc.tile_pool(name="outp", bufs=3) as outp,
                tc.tile_pool(name="eps", bufs=8) as eps,
            ):
                idx_sb = idxp.tile([P, ncalls * IDW16], dt.int16, name="idx_sb")
                nc.sync.dma_start(out=idx_sb[:], in_=IDX.ap()[:, :])

                acc = {}     # kn tile -> acc sbuf tile
                cur_ps = {}  # (graph,tile) currently accumulating
                n_kn_graphs = cfg["n_kn_graphs"]

                def epilogue(gi, t, ps):
                    den = eps.tile([P, 1], dt.float32, tag="den", name=f"den{gi}_{t}")
                    z0 = eps.tile([P, 1], dt.float32, tag="z0", name=f"z0{gi}_{t}")
                    nc.vector.tensor_scalar(z0[:, :1], ps[:, d_out:d_out + 1],
                                            0.0, None, AOT.is_equal)
                    nc.vector.tensor_tensor(out=den[:, :1], in0=ps[:, d_out:d_out + 1],
                                            in1=z0[:, :1], op=AOT.add)
                    rec = eps.tile([P, 1], dt.float32, tag="rec", name=f"rec{gi}_{t}")
                    nc.vector.reciprocal(rec[:, :1], den[:, :1])
                    if gi == 0:
                        a = accp.tile([P, d_out], dt.float32, tag="acc", name=f"acc{t}")
                        acc[t] = a
                        nc.vector.tensor_scalar(a[:], ps[:, 0:d_out], rec[:, :1],
                                                None, AOT.mult)
                    elif gi < n_kn_graphs:
                        tmp = outp.tile([P, d_out], dt.float32, tag="o", name=f"tmp{gi}_{t}")
                        nc.vector.tensor_scalar(tmp[:], ps[:, 0:d_out], rec[:, :1],
                                                None, AOT.mult)
                        nc.vector.tensor_tensor(out=acc[t][:], in0=acc[t][:],
                                                in1=tmp[:], op=AOT.add)
                        if gi == n_kn_graphs - 1:
                            nc.sync.dma_start(out=KN_OUT.ap()[t * P:(t + 1) * P, :],
                                              in_=acc[t][:])
                    else:
                        o = outp.tile([P, d_out], dt.float32, tag="o", name=f"o{t}")
                        nc.vector.tensor_scalar(o[:], ps[:, 0:d_out], rec[:, :1],
                                                None, AOT.mult)
                        nc.sync.dma_start(out=EX_OUT.ap()[t * P:(t + 1) * P, :], in_=o[:])

                gt_tiles, oh_tiles = {}, {}
                for ev in cfg["sched"]:
                    if ev[0] == "g":
                        call = ev[1]
                        gt = gp.tile([P, CPC, TW], dt.bfloat16, tag="gt",
                                     name=f"gt{call}")
                        nc.gpsimd.dma_gather(
                            gt[:], tables[call_tab[call]][:, :],
                            idx_sb[:, call * IDW16:(call + 1) * IDW16],
                            IDW, IDW, TW,
                        )
                        oh_sb = ohp.tile([P, CPC * P], dt.bfloat16, tag="oh",
                                         name=f"oh{call}")
                        nc.sync.dma_start(out=oh_sb[:], in_=OH.ap()[call])
                        gt_tiles[call], oh_tiles[call] = gt, oh_sb
                    else:
                        _, call, slot, gi, t, start, stop = ev
                        if start:
                            ps = psB.tile([P, TW], dt.float32, tag="psB",
                                          name=f"psB{gi}_{t}")
                            cur_ps[(gi, t)] = ps
                        ps = cur_ps[(gi, t)]
                        gt, oh_sb = gt_tiles[call], oh_tiles[call]
                        for (s0, s1) in NSL:
                            nc.tensor.matmul(
                                out=ps[:, s0:s1],
                                lhsT=oh_sb[:, slot * P:(slot + 1) * P],
                                rhs=gt[:, slot, s0:s1],
                                start=start, stop=stop,
                            )
                        if stop:
                            epilogue(gi, t, ps)

    nc.compile()
    return nc


# ----------------------------------------------------------------------------
# Entry point
# ----------------------------------------------------------------------------

_CACHE = {}


def _run(inputs, R=8, sim=False):
    cfg, in_maps, meta = _prepare(inputs, R=R)

    key = (R, cfg["ncalls"], cfg["NBAND"], cfg["NBLK"],
           cfg["SH_ROWS"], cfg["TW"], cfg["KC"],
           tuple(tuple(e) for e in cfg["sched"]), sim)
    if key in _CACHE:
        nc = _CACHE[key]
    else:
        nc = _build(cfg, debug=sim, asserts=sim)
        _CACHE[key] = nc

    if sim:
        from concourse.bass_interp import MultiCoreSim
        msim = MultiCoreSim(nc, num_cores=R)
        for c in range(R):
            for k, v in in_maps[c].items():
                msim.cores[c].tensor(k)[:] = v
        msim.simulate(check_with_hw=False)
        results = [
            {"kn_out": np.array(msim.cores[c].tensor("kn_out")),
             "ex_out": np.array(msim.cores[c].tensor("ex_out"))}
            for c in range(R)
        ]
        exec_ns = None
    else:
        from concourse.bass_utils import run_bass_kernel_spmd
        trace = bool(int(os.environ.get("KERNEL_TRACE", "0")))
        br = run_bass_kernel_spmd(nc, in_maps, list(range(R)), trace=trace)
        results = br.results
        exec_ns = br.exec_time_ns

    n_k, n_e, d_out = meta["n_k"], meta["n_e"], meta["d_out"]
    kn_out = np.concatenate([results[c]["kn_out"] for c in range(R)], axis=0)
    ex_out = np.concatenate([results[c]["ex_out"] for c in range(R)], axis=0)
    assert kn_out.shape == (n_k, d_out) and ex_out.shape == (n_e, d_out)
    return (np.asarray(kn_out, np.float32), np.asarray(ex_out, np.float32)), exec_ns


def kernel(**inputs):
    out, _ = _run(inputs, R=8, sim=False)
    return out


def kernel_timed(**inputs):
    return _run(inputs, R=8, sim=False)


def kernel_sim(R=2, **inputs):
    out, _ = _run(inputs, R=R, sim=True)
    return out
